# revision 46
# baseline (speedup 1.0000x reference)
"""DeepSeek-MoE (16 routed experts, top-2, 1 shared expert) on 8 Trainium2 cores.

Default strategy "ep2" (expert-parallel, two launches):
  Stage 1 (data-parallel over tokens): each core takes 1024 of the 8192
  tokens and computes the router (exact fp32), top-2 gates, balance-loss
  stats, plus the shared expert and residual:  y1 = x + shared(x) +
  sum_e g_e*rb2[e] + sb2.  It returns the full gate matrix.
  Host dispatch: tokens are packed per expert from the device-computed
  top-k gates (the "all-to-all" of the sharding hint, done host-side since
  the kernel contract is full-input/full-output).
  Stage 2 (expert-parallel): core c holds routed experts 2c and 2c+1 and
  runs them densely over just their assigned (padded) tokens, applying the
  gate to the silu'd intermediate before the second matmul. Host adds the
  gathered expert outputs into y1.

Big matmuls run as float32r (full-rate fp32 PE mode, ~1e-4 rel err); the
router runs exact fp32 so top-2 selection matches the reference.

Fallback strategy "dense" (MOE_STRATEGY=dense): single launch, each core
computes all 16 experts densely for its token shard.

Shapes hardcoded per the problem spec: hidden_states [4, 2048, 1024],
E=16, L=512, H=1024, top-2.
"""

import os
import sys

for _p in ("/opt/trn_rl_repo", "/root/.axon_site/_ro/trn_rl_repo"):
    if _p not in sys.path:
        sys.path.append(_p)

import numpy as np
from contextlib import ExitStack

import concourse.bass as bass
import concourse.tile as tile
from concourse import bacc, mybir
from concourse.bass_utils import run_bass_kernel_spmd
from concourse.masks import make_identity

F32 = mybir.dt.float32
F32R = mybir.dt.float32r
AF = mybir.ActivationFunctionType
ALU = mybir.AluOpType

P = 128
B, S, H, L, E = 4, 2048, 1024, 512, 16
TOP_K = 2
BALANCE_ALPHA = 0.001
N_CORES = 8
T_FULL = B * S
T = T_FULL // N_CORES  # tokens per core in stage 1 (1024)
HC = H // P  # 8 contraction chunks over H
LT = L // P  # 4 tiles over L
HT = H // P  # 8 tiles over H
NH = T // 512  # moving-dim halves (2)
EPC = E // N_CORES  # routed experts per core in stage 2 (2)


def _col_tiles(c):
    """Split token-column count c into moving-dim tiles of <=512."""
    out, o = [], 0
    while o < c:
        w = min(512, c - o)
        out.append((o, w))
        o += w
    return out


def _router_and_gates(nc, tc, ctx, pools, xt_sb, ct_sb, eb_sb, ones_col,
                      identity, gT, stats_ap):
    """Emit router matmuls (exact fp32), top-2 gating, and balance stats.

    Fills gT [E, T] in SBUF and DMAs stats [E, 2] (mask row-sums, s' row-
    sums) to DRAM.
    """
    small, pm_pool, ps_pool = pools
    aff_T = ctx.enter_context(tc.tile_pool(name="aff", bufs=1))
    aff = aff_T.tile([E, T], F32)
    biased = aff_T.tile([E, T], F32) if eb_sb is not None else aff

    for half in range(NH):
        pa = pm_pool.tile([E, 512], F32, tag="pt")
        for hc in range(HC):
            nc.tensor.matmul(pa[:], ct_sb[:, hc, :],
                             xt_sb[:, hc, half * 512:(half + 1) * 512],
                             start=(hc == 0), stop=(hc == HC - 1))
        nc.scalar.activation(aff[:, half * 512:(half + 1) * 512], pa[:],
                             AF.Sigmoid)
    if eb_sb is not None:
        nc.vector.tensor_scalar_add(biased[:], aff[:], eb_sb[:, 0:1])

    pmask_sum = ps_pool.tile([E, 1], F32)
    psp_sum = ps_pool.tile([E, 1], F32)
    for tt in range(T // P):
        tsl = slice(tt * P, (tt + 1) * P)
        pta = pm_pool.tile([P, E], F32, tag="pt")
        nc.tensor.transpose(pta[:], aff[:, tsl], identity[0:E, 0:E])
        afftok = small.tile([P, E], F32)
        nc.vector.tensor_copy(afftok[:], pta[:])
        if eb_sb is not None:
            ptb = pm_pool.tile([P, E], F32, tag="pt")
            nc.tensor.transpose(ptb[:], biased[:, tsl], identity[0:E, 0:E])
            biastok = small.tile([P, E], F32)
            nc.vector.tensor_copy(biastok[:], ptb[:])
        else:
            biastok = afftok

        mx8 = small.tile([P, 8], F32)
        nc.vector.max(mx8[:], biastok[:])
        mask = small.tile([P, E], F32)
        nc.vector.tensor_scalar(mask[:], biastok[:], mx8[:, 1:2], None,
                                op0=ALU.is_ge)
        sel = small.tile([P, E], F32)
        nc.vector.tensor_mul(sel[:], afftok[:], mask[:])
        den = small.tile([P, 1], F32)
        nc.vector.reduce_sum(den[:], sel[:], axis=mybir.AxisListType.X)
        nc.vector.tensor_scalar_add(den[:], den[:], 1e-8)
        rec = small.tile([P, 1], F32)
        nc.vector.reciprocal(rec[:], den[:])
        g_tok = small.tile([P, E], F32)
        nc.vector.tensor_scalar_mul(g_tok[:], sel[:], rec[:, 0:1])

        den2 = small.tile([P, 1], F32)
        nc.vector.reduce_sum(den2[:], afftok[:], axis=mybir.AxisListType.X)
        nc.vector.tensor_scalar_add(den2[:], den2[:], 1e-8)
        rec2 = small.tile([P, 1], F32)
        nc.vector.reciprocal(rec2[:], den2[:])
        sprime = small.tile([P, E], F32)
        nc.vector.tensor_scalar_mul(sprime[:], afftok[:], rec2[:, 0:1])

        nc.tensor.matmul(pmask_sum[:], mask[:], ones_col[:],
                         start=(tt == 0), stop=(tt == T // P - 1))
        nc.tensor.matmul(psp_sum[:], sprime[:], ones_col[:],
                         start=(tt == 0), stop=(tt == T // P - 1))

        ptg = pm_pool.tile([E, P], F32, tag="pt")
        nc.tensor.transpose(ptg[:], g_tok[:], identity[:])
        nc.vector.tensor_copy(gT[:, tsl], ptg[:])

    stats_sb = small.tile([E, 2], F32)
    nc.vector.tensor_copy(stats_sb[:, 0:1], pmask_sum[:])
    nc.vector.tensor_copy(stats_sb[:, 1:2], psp_sum[:])
    nc.sync.dma_start(stats_ap[:], stats_sb[:])


def _build_stage1(zero_bias=True):
    """Router + gates + stats + shared expert + residual, DP over tokens.

    zero_bias=True omits the expert_biases add and the rb2/sb2 correction
    (all zero in this model); the False variant keeps the exact math.
    """
    nc = bacc.Bacc("TRN2", target_bir_lowering=False, debug=False,
                   num_devices=N_CORES)

    xta = nc.dram_tensor("xta", [P, 3, T], F32, kind="ExternalInput").ap()
    xtb = nc.dram_tensor("xtb", [P, 3, T], F32, kind="ExternalInput").ap()
    xtc = nc.dram_tensor("xtc", [P, 2, T], F32, kind="ExternalInput").ap()
    ct = nc.dram_tensor("ct", [P, HC, E], F32, kind="ExternalInput").ap()
    s1 = nc.dram_tensor("s1", [P, HC, L], F32, kind="ExternalInput").ap()
    s2 = nc.dram_tensor("s2", [P, LT, H], F32, kind="ExternalInput").ap()
    sb1 = nc.dram_tensor("sb1", [P, LT], F32, kind="ExternalInput").ap()
    if not zero_bias:
        eb = nc.dram_tensor("eb", [E, 1], F32, kind="ExternalInput").ap()
        rb2 = nc.dram_tensor("rb2", [E + 1, H], F32, kind="ExternalInput").ap()
    yt = nc.dram_tensor("yt", [H, T], F32, kind="ExternalOutput").ap()
    gt_out = nc.dram_tensor("gt_out", [E, T], F32, kind="ExternalOutput").ap()
    stats = nc.dram_tensor("stats", [E, 2], F32, kind="ExternalOutput").ap()

    with tile.TileContext(nc) as tc, ExitStack() as ctx:
        const = ctx.enter_context(tc.tile_pool(name="const", bufs=1))
        xpool = ctx.enter_context(tc.tile_pool(name="xpool", bufs=1))
        work = ctx.enter_context(tc.tile_pool(name="work", bufs=1))
        small = ctx.enter_context(tc.tile_pool(name="small", bufs=4))
        ph_pool = ctx.enter_context(tc.tile_pool(name="ph", bufs=2, space="PSUM"))
        po_pool = ctx.enter_context(tc.tile_pool(name="po", bufs=2, space="PSUM"))
        pm_pool = ctx.enter_context(tc.tile_pool(name="pm", bufs=2, space="PSUM"))
        ps_pool = ctx.enter_context(tc.tile_pool(name="ps", bufs=1, space="PSUM"))

        identity = const.tile([P, P], F32)
        make_identity(nc, identity)
        ones_col = const.tile([P, 1], F32)
        nc.any.memset(ones_col[:], 1.0)

        # PE warm-up: a few throwaway matmuls during the DMA lead push the
        # HAM clock gate to full rate before the fp32 router matmuls issue
        warm = const.tile([P, 512], F32)
        nc.any.memset(warm[:], 0.0)
        pwarm = pm_pool.tile([P, 512], F32, tag="pt")
        for _ in range(4):
            nc.tensor.matmul(pwarm[:], warm[:, 0:P], warm[:],
                             start=True, stop=True)

        # x (exact fp32) split across both HW DGE queues (SP + ACT);
        # each piece is partition-contiguous in DRAM
        xt_sb = xpool.tile([P, HC, T], F32, tag="x")
        nc.sync.dma_start(xt_sb[:, 0:3, :], xta[:])
        nc.scalar.dma_start(xt_sb[:, 3:6, :], xtb[:])
        nc.sync.dma_start(xt_sb[:, 6:8, :], xtc[:])
        ct_sb = const.tile([P, HC, E], F32)
        nc.scalar.dma_start(ct_sb[:], ct[:])
        sb1_sb = const.tile([P, LT], F32)
        nc.sync.dma_start(sb1_sb[:], sb1[:])

        # shared-expert operands stream on the gpsimd (casting) queue in
        # parallel with the fp32 x load: w1 then per-chunk f32r x so the
        # shared expert's first matmuls start before the router's x lands
        w1_sb = work.tile([P, HC, L], F32R, tag="w1")
        for q in range(4):
            nc.gpsimd.dma_start(w1_sb[:, 2 * q:2 * q + 2, :],
                                s1[:, 2 * q:2 * q + 2, :])
        xt_r = [const.tile([P, T], F32R, name=f"xtr{hc}") for hc in range(HC)]
        for hc in range(HC):
            if hc < 3:
                nc.gpsimd.dma_start(xt_r[hc][:], xta[:, hc, :])
            elif hc < 6:
                nc.gpsimd.dma_start(xt_r[hc][:], xtb[:, hc - 3, :])
            else:
                nc.gpsimd.dma_start(xt_r[hc][:], xtc[:, hc - 6, :])
        w2_sb = work.tile([P, LT, H], F32R, tag="w2")
        for q in range(4):
            nc.gpsimd.dma_start(w2_sb[:, q, :], s2[:, q, :])

        acc = const.tile([P, HT, T], F32)
        halves = [slice(h * 512, (h + 1) * 512) for h in range(NH)]

        # shared-expert first matmul chain emitted BEFORE the router: its
        # f32r operands stream on gpsimd while the exact-fp32 x is still
        # loading, so the PE has work during the router's DMA wait
        z_sb = work.tile([P, LT, T], F32R, tag="z")
        for lt in range(LT):
            lsl = slice(lt * P, (lt + 1) * P)
            phts = [ph_pool.tile([P, 512], F32, tag="ph", name=f"ph{lt}_{h}")
                    for h in range(NH)]
            for hc in range(HC):
                for h, csl in enumerate(halves):
                    nc.tensor.matmul(phts[h][:], w1_sb[:, hc, lsl],
                                     xt_r[hc][:, csl],
                                     start=(hc == 0), stop=(hc == HC - 1))
            for h, csl in enumerate(halves):
                nc.scalar.activation(z_sb[:, lt, csl], phts[h][:], AF.Silu,
                                     bias=sb1_sb[:, lt:lt + 1])

        if zero_bias:
            gT = const.tile([E, T], F32)
            eb_sb = None
        else:
            eb_sb = const.tile([E, 1], F32)
            nc.sync.dma_start(eb_sb[:], eb[:])
            rb2_sb = const.tile([E + 1, H], F32)
            nc.sync.dma_start(rb2_sb[:], rb2[:])
            gTx = const.tile([E + 1, T], F32)
            gT = gTx[0:E, :]
            # row E stays 1.0 (gate for the sb2 bias row)
            nc.any.memset(gTx[:], 1.0)

        _router_and_gates(nc, tc, ctx, (small, pm_pool, ps_pool), xt_sb,
                          ct_sb, eb_sb, ones_col, identity, gT, stats)
        nc.sync.dma_start(gt_out[:], gT[:])

        if not zero_bias:
            # acc = x + sum_e g_e*rb2[e] + sb2   (sb2 rides as gate-1 row E)
            for ht in range(HT):
                hsl = slice(ht * P, (ht + 1) * P)
                for half in range(NH):
                    csl = slice(half * 512, (half + 1) * 512)
                    pc = po_pool.tile([P, 512], F32, tag="po")
                    nc.tensor.matmul(pc[:], rb2_sb[:, hsl], gTx[:, csl],
                                     start=True, stop=True)
                    nc.vector.tensor_add(acc[:, ht, csl], xt_sb[:, ht, csl],
                                         pc[:])

        for ht in range(HT):
            hsl = slice(ht * P, (ht + 1) * P)
            pots = [po_pool.tile([P, 512], F32, tag="po", name=f"po{ht}_{h}")
                    for h in range(NH)]
            for lc in range(LT):
                for h, csl in enumerate(halves):
                    nc.tensor.matmul(pots[h][:], w2_sb[:, lc, hsl],
                                     z_sb[:, lc, csl],
                                     start=(lc == 0), stop=(lc == LT - 1))
            src_ap = xt_sb if zero_bias else acc
            for h, csl in enumerate(halves):
                nc.vector.tensor_add(acc[:, ht, csl], src_ap[:, ht, csl],
                                     pots[h][:])

        # stores split across both HW DGE queues
        for ht in range(HT):
            eng = nc.sync if ht % 2 == 0 else nc.scalar
            eng.dma_start(yt[ht * P:(ht + 1) * P, :], acc[:, ht, :])

    nc.compile()
    return nc


def _build_stage2(C1, C2):
    """Two routed experts per core over their gathered (padded) tokens.

    Slot 0 holds a high-count expert (capacity C1), slot 1 a low-count
    one (C2 <= C1) — the host ranks experts by load so padding is small.
    """
    nc = bacc.Bacc("TRN2", target_bir_lowering=False, debug=False,
                   num_devices=N_CORES)

    caps = (C1, C2)
    xes = [nc.dram_tensor(f"xe{k}", [P, HC, caps[k]], F32,
                          kind="ExternalInput").ap() for k in range(EPC)]
    ges = [nc.dram_tensor(f"ge{k}", [1, caps[k]], F32,
                          kind="ExternalInput").ap() for k in range(EPC)]
    w1p = nc.dram_tensor("w1p", [EPC, P, HC, L], F32, kind="ExternalInput").ap()
    w2p = nc.dram_tensor("w2p", [EPC, P, LT, H], F32, kind="ExternalInput").ap()
    rb1p = nc.dram_tensor("rb1p", [EPC, P, LT], F32, kind="ExternalInput").ap()
    yes = [nc.dram_tensor(f"ye{k}", [H, caps[k]], F32,
                          kind="ExternalOutput").ap() for k in range(EPC)]

    with tile.TileContext(nc) as tc, ExitStack() as ctx:
        const = ctx.enter_context(tc.tile_pool(name="const", bufs=1))
        xep = ctx.enter_context(tc.tile_pool(name="xep", bufs=16))
        xw = ctx.enter_context(tc.tile_pool(name="xw", bufs=2))
        zw = ctx.enter_context(tc.tile_pool(name="zw", bufs=1))
        small = ctx.enter_context(tc.tile_pool(name="small", bufs=4))
        ph_pool = ctx.enter_context(tc.tile_pool(name="ph", bufs=3, space="PSUM"))
        po_pool = ctx.enter_context(tc.tile_pool(name="po", bufs=3, space="PSUM"))
        pm_pool = ctx.enter_context(tc.tile_pool(name="pm", bufs=2, space="PSUM"))

        ones_row = const.tile([1, P], F32)
        nc.any.memset(ones_row[:], 1.0)
        rb1_sb = const.tile([P, EPC, LT], F32)
        nc.sync.dma_start(rb1_sb[:], rb1p.rearrange("k p l -> p k l"))

        warm = const.tile([P, 512], F32)
        nc.any.memset(warm[:], 0.0)
        pwarm = pm_pool.tile([P, 512], F32, tag="pt")
        for _ in range(4):
            nc.tensor.matmul(pwarm[:], warm[:, 0:P], warm[:],
                             start=True, stop=True)

        # both gate rows up front: tiny loads that must not queue behind
        # expert 0's store traffic on the sync queue (expert 1's gb-broadcast
        # matmuls sit ahead of its expert matmuls in the in-order PE stream)
        g_rows = [xw.tile([1, caps[k]], F32, tag="g_row", name=f"g_row{k}")
                  for k in range(EPC)]
        for k in range(EPC):
            nc.sync.dma_start(g_rows[k][:], ges[k])

        for k in range(EPC):
            C = caps[k]
            ctiles = _col_tiles(C)
            w1_sb = xw.tile([P, HC, L], F32R, tag="w1")
            for q in range(4):
                nc.gpsimd.dma_start(w1_sb[:, 2 * q:2 * q + 2, :],
                                    w1p[k][:, 2 * q:2 * q + 2, :])
            # per-chunk token tiles: lets expert k+1's loads start while
            # expert k's first matmul chain is still reading its chunks
            xe_r = [xep.tile([P, C], F32R, tag="xec", name=f"xec{k}_{hc}")
                    for hc in range(HC)]
            for hc in range(HC):
                nc.gpsimd.dma_start(xe_r[hc][:], xes[k][:, hc, :])
            w2_sb = xw.tile([P, LT, H], F32R, tag="w2")
            for q in range(4):
                nc.gpsimd.dma_start(w2_sb[:, q, :], w2p[k][:, q, :])

            g_row = g_rows[k]
            gb_sb = xw.tile([P, C], F32, tag="gb")
            for co, cw in ctiles:
                pgb = pm_pool.tile([P, 512], F32, tag="pt")
                nc.tensor.matmul(pgb[:, :cw], ones_row[:], g_row[0:1, co:co + cw],
                                 start=True, stop=True)
                nc.vector.tensor_copy(gb_sb[:, co:co + cw], pgb[:, :cw])

            # group col tiles in pairs so each weight load serves 2 matmuls
            ctpairs = [ctiles[i:i + 2] for i in range(0, len(ctiles), 2)]
            z_sb = zw.tile([P, LT, C], F32R, tag="z")
            for lt in range(LT):
                lsl = slice(lt * P, (lt + 1) * P)
                for pi, pair in enumerate(ctpairs):
                    phts = [ph_pool.tile([P, 512], F32, tag="ph",
                                         name=f"ph{k}_{lt}_{pi}_{j}")
                            for j in range(len(pair))]
                    for hc in range(HC):
                        for j, (co, cw) in enumerate(pair):
                            nc.tensor.matmul(
                                phts[j][:, :cw], w1_sb[:, hc, lsl],
                                xe_r[hc][:, co:co + cw],
                                start=(hc == 0), stop=(hc == HC - 1))
                    for j, (co, cw) in enumerate(pair):
                        csl = slice(co, co + cw)
                        nc.scalar.activation(z_sb[:, lt, csl], phts[j][:, :cw],
                                             AF.Silu,
                                             bias=rb1_sb[:, k, lt:lt + 1])
                        nc.vector.tensor_mul(z_sb[:, lt, csl],
                                             z_sb[:, lt, csl], gb_sb[:, csl])

            for ht in range(HT):
                hsl = slice(ht * P, (ht + 1) * P)
                for pi, pair in enumerate(ctpairs):
                    pots = [po_pool.tile([P, 512], F32, tag="po",
                                         name=f"po{k}_{ht}_{pi}_{j}")
                            for j in range(len(pair))]
                    for lc in range(LT):
                        for j, (co, cw) in enumerate(pair):
                            nc.tensor.matmul(
                                pots[j][:, :cw], w2_sb[:, lc, hsl],
                                z_sb[:, lc, co:co + cw],
                                start=(lc == 0), stop=(lc == LT - 1))
                    for j, (co, cw) in enumerate(pair):
                        csl = slice(co, co + cw)
                        stg = small.tile([P, 512], F32, tag="stg")
                        nc.vector.tensor_copy(stg[:, :cw], pots[j][:, :cw])
                        eng = nc.sync if (ht + pi + j) % 2 == 0 else nc.scalar
                        eng.dma_start(yes[k][ht * P:(ht + 1) * P, csl],
                                      stg[:, :cw])

    nc.compile()
    return nc


_NC_CACHE = {}


def _get(name, builder, *args):
    key = (name,) + args
    if key not in _NC_CACHE:
        _NC_CACHE[key] = builder(*args)
    return _NC_CACHE[key]


def _prep_host(inputs):
    (hidden_states, expert_centroids, expert_biases, sw1, sb1, sw2, sb2,
     rw1, rb1, rw2, rb2) = inputs
    flat = hidden_states.reshape(T_FULL, H)
    prep = {
        "flat": flat,
        "ct": np.ascontiguousarray(expert_centroids.T),
        "eb": np.ascontiguousarray(expert_biases.reshape(E, 1)),
        "s1": np.ascontiguousarray(sw1.sum(axis=0)),
        "s2": np.ascontiguousarray(sw2.sum(axis=0)),
        "sb1": np.ascontiguousarray(sb1.sum(axis=0).reshape(LT, P).T),
        "sb2": np.ascontiguousarray(sb2.sum(axis=0).reshape(HT, P).T),
        "rb1": np.ascontiguousarray(rb1.reshape(E, LT, P).transpose(0, 2, 1)),
        "rb2": rb2, "rw1": rw1, "rw2": rw2,
        "rb2x": np.ascontiguousarray(
            np.concatenate([rb2, sb2.sum(axis=0)[None]], axis=0)),
    }
    return prep


def _aux_from_stats(mask_sum, sp_sum):
    f_i = mask_sum * (E / (TOP_K * S)) / B
    p_i = sp_sum / T_FULL
    return np.float32(BALANCE_ALPHA * float((f_i * p_i).sum()))


def _perm_h(w):
    """[H, N] -> partition-major [P, HC, N] (contiguous per partition)."""
    return np.ascontiguousarray(w.reshape(-1, P, w.shape[-1]).transpose(1, 0, 2))


def _kernel_ep2(prep):
    zero_bias = (not prep["eb"].any()) and (not prep["rb2x"].any())
    nc1 = _get("s1", _build_stage1, zero_bias)
    flatT_perm = _perm_h(np.ascontiguousarray(prep["flat"].T))  # [P, HC, TF]
    ct_p = _perm_h(prep["ct"])
    s1_p = _perm_h(prep["s1"])
    s2_p = _perm_h(prep["s2"])
    in1 = []
    for c in range(N_CORES):
        xt_c = flatT_perm[:, :, c * T:(c + 1) * T]
        m = {"xta": np.ascontiguousarray(xt_c[:, 0:3]),
             "xtb": np.ascontiguousarray(xt_c[:, 3:6]),
             "xtc": np.ascontiguousarray(xt_c[:, 6:8]),
             "ct": ct_p, "s1": s1_p, "s2": s2_p, "sb1": prep["sb1"]}
        if not zero_bias:
            m["eb"] = prep["eb"]
            m["rb2"] = prep["rb2x"]
        in1.append(m)
    res1 = run_bass_kernel_spmd(nc1, in1, core_ids=list(range(N_CORES)))

    mask_sum = np.zeros(E, dtype=np.float64)
    sp_sum = np.zeros(E, dtype=np.float64)
    gT_full = np.empty((E, T_FULL), dtype=np.float32)
    out = np.empty((T_FULL, H), dtype=np.float32)
    for c in range(N_CORES):
        r = res1.results[c]
        out[c * T:(c + 1) * T] = r["yt"].T
        gT_full[:, c * T:(c + 1) * T] = r["gt_out"]
        mask_sum += r["stats"][:, 0]
        sp_sum += r["stats"][:, 1]
    aux = _aux_from_stats(mask_sum, sp_sum)

    # token dispatch on the device-computed top-k gates; rank experts by
    # load so the low-count slot can use a smaller capacity
    idx = [np.nonzero(gT_full[e] > 0)[0] for e in range(E)]
    order = sorted(range(E), key=lambda e: -len(idx[e]))
    # core c: slot 0 <- rank c (heavy), slot 1 <- rank 15-c (light)
    assign = [(order[c], order[E - 1 - c]) for c in range(N_CORES)]
    rup = lambda n: max(512, -(-n // 256) * 256)
    C1 = rup(max(len(idx[e]) for e, _ in assign))
    C2 = rup(max(len(idx[e]) for _, e in assign))

    w1_perm = np.ascontiguousarray(
        prep["rw1"].reshape(E, HC, P, L).transpose(0, 2, 1, 3))
    w2_perm = np.ascontiguousarray(
        prep["rw2"].reshape(E, LT, P, H).transpose(0, 2, 1, 3))

    nc2 = _get("s2", _build_stage2, C1, C2)
    caps = (C1, C2)
    in2 = []
    for c in range(N_CORES):
        m = {}
        for k in range(EPC):
            e = assign[c][k]
            n = len(idx[e])
            xe = np.zeros((P, HC, caps[k]), dtype=np.float32)
            gevals = np.zeros((1, caps[k]), dtype=np.float32)
            xe[:, :, :n] = flatT_perm[:, :, idx[e]]
            gevals[0, :n] = gT_full[e, idx[e]]
            m[f"xe{k}"] = xe
            m[f"ge{k}"] = gevals
        es = list(assign[c])
        m["w1p"] = np.ascontiguousarray(w1_perm[es])
        m["w2p"] = np.ascontiguousarray(w2_perm[es])
        m["rb1p"] = np.ascontiguousarray(prep["rb1"][es])
        in2.append(m)
    res2 = run_bass_kernel_spmd(nc2, in2, core_ids=list(range(N_CORES)))

    for c in range(N_CORES):
        for k in range(EPC):
            e = assign[c][k]
            n = len(idx[e])
            out[idx[e]] += res2.results[c][f"ye{k}"][:, :n].T
    return out.reshape(B, S, H), aux


def _kernel_dense(prep):
    nc = _get("dense", _build_dense)
    in_maps = []
    for c in range(N_CORES):
        xt_c = np.ascontiguousarray(prep["flat"][c * T:(c + 1) * T].T)
        in_maps.append({
            "xt": xt_c, "ct": prep["ct"], "eb": prep["eb"],
            "w1": prep["rw1"], "w2": prep["rw2"], "s1": prep["s1"],
            "s2": prep["s2"], "rb1": prep["rb1"], "rb2": prep["rb2"],
            "sb1": prep["sb1"], "sb2": prep["sb2"],
        })
    res = run_bass_kernel_spmd(nc, in_maps, core_ids=list(range(N_CORES)))

    out = np.empty((T_FULL, H), dtype=np.float32)
    mask_sum = np.zeros(E, dtype=np.float64)
    sp_sum = np.zeros(E, dtype=np.float64)
    for c in range(N_CORES):
        out[c * T:(c + 1) * T] = res.results[c]["yt"].T
        st = res.results[c]["stats"]
        mask_sum += st[:, 0]
        sp_sum += st[:, 1]
    return out.reshape(B, S, H), _aux_from_stats(mask_sum, sp_sum)


def _build_dense():
    """Single-launch fallback: every core runs all experts on its shard."""
    nc = bacc.Bacc("TRN2", target_bir_lowering=False, debug=False,
                   num_devices=N_CORES)

    xt = nc.dram_tensor("xt", [H, T], F32, kind="ExternalInput").ap()
    ct = nc.dram_tensor("ct", [H, E], F32, kind="ExternalInput").ap()
    eb = nc.dram_tensor("eb", [E, 1], F32, kind="ExternalInput").ap()
    w1 = nc.dram_tensor("w1", [E, H, L], F32, kind="ExternalInput").ap()
    w2 = nc.dram_tensor("w2", [E, L, H], F32, kind="ExternalInput").ap()
    s1 = nc.dram_tensor("s1", [H, L], F32, kind="ExternalInput").ap()
    s2 = nc.dram_tensor("s2", [L, H], F32, kind="ExternalInput").ap()
    rb1 = nc.dram_tensor("rb1", [E, P, LT], F32, kind="ExternalInput").ap()
    rb2 = nc.dram_tensor("rb2", [E, H], F32, kind="ExternalInput").ap()
    sb1 = nc.dram_tensor("sb1", [P, LT], F32, kind="ExternalInput").ap()
    sb2 = nc.dram_tensor("sb2", [P, HT], F32, kind="ExternalInput").ap()
    yt = nc.dram_tensor("yt", [H, T], F32, kind="ExternalOutput").ap()
    stats = nc.dram_tensor("stats", [E, 2], F32, kind="ExternalOutput").ap()

    with tile.TileContext(nc) as tc, ExitStack() as ctx:
        const = ctx.enter_context(tc.tile_pool(name="const", bufs=1))
        xpool = ctx.enter_context(tc.tile_pool(name="xpool", bufs=1))
        work = ctx.enter_context(tc.tile_pool(name="work", bufs=2))
        small = ctx.enter_context(tc.tile_pool(name="small", bufs=4))
        ph_pool = ctx.enter_context(tc.tile_pool(name="ph", bufs=2, space="PSUM"))
        po_pool = ctx.enter_context(tc.tile_pool(name="po", bufs=2, space="PSUM"))
        pm_pool = ctx.enter_context(tc.tile_pool(name="pm", bufs=2, space="PSUM"))
        ps_pool = ctx.enter_context(tc.tile_pool(name="ps", bufs=1, space="PSUM"))

        identity = const.tile([P, P], F32)
        make_identity(nc, identity)
        ones_col = const.tile([P, 1], F32)
        nc.any.memset(ones_col[:], 1.0)
        ones_row = const.tile([1, P], F32)
        nc.any.memset(ones_row[:], 1.0)

        xt_sb = xpool.tile([P, HC, T], F32, tag="x")
        nc.sync.dma_start(xt_sb[:], xt.rearrange("(c p) t -> p c t", p=P))
        ct_sb = const.tile([P, HC, E], F32)
        nc.sync.dma_start(ct_sb[:], ct.rearrange("(c p) e -> p c e", p=P))
        eb_sb = const.tile([E, 1], F32)
        nc.sync.dma_start(eb_sb[:], eb[:])
        rb2_sb = const.tile([E, H], F32)
        nc.sync.dma_start(rb2_sb[:], rb2[:])
        rb1_sb = const.tile([P, E, LT], F32)
        nc.sync.dma_start(rb1_sb[:], rb1.rearrange("e p l -> p e l"))
        sb1_sb = const.tile([P, LT], F32)
        nc.sync.dma_start(sb1_sb[:], sb1[:])
        sb2_sb = const.tile([P, HT], F32)
        nc.sync.dma_start(sb2_sb[:], sb2[:])

        acc = const.tile([P, HT, T], F32)
        gT = const.tile([E, T], F32)

        _router_and_gates(nc, tc, ctx, (small, pm_pool, ps_pool), xt_sb,
                          ct_sb, eb_sb, ones_col, identity, gT, stats)

        for ht in range(HT):
            hsl = slice(ht * P, (ht + 1) * P)
            for half in range(NH):
                csl = slice(half * 512, (half + 1) * 512)
                pc = po_pool.tile([P, 512], F32, tag="po")
                nc.tensor.matmul(pc[:], rb2_sb[:, hsl], gT[:, csl],
                                 start=True, stop=True)
                nc.vector.tensor_add(acc[:, ht, csl], xt_sb[:, ht, csl], pc[:])
                nc.vector.tensor_scalar_add(acc[:, ht, csl], acc[:, ht, csl],
                                            sb2_sb[:, ht:ht + 1])

        xt_r = xpool.tile([P, HC, T], F32R, tag="x")
        nc.gpsimd.dma_start(xt_r[:], xt.rearrange("(c p) t -> p c t", p=P))

        for e in range(E + 1):
            shared = (e == E)
            w1_sb = work.tile([P, HC, L], F32R, tag="w1")
            w2_sb = work.tile([P, LT, H], F32R, tag="w2")
            if shared:
                nc.gpsimd.dma_start(w1_sb[:], s1.rearrange("(c p) l -> p c l", p=P))
                nc.gpsimd.dma_start(w2_sb[:], s2.rearrange("(c p) h -> p c h", p=P))
            else:
                nc.gpsimd.dma_start(w1_sb[:], w1[e].rearrange("(c p) l -> p c l", p=P))
                nc.gpsimd.dma_start(w2_sb[:], w2[e].rearrange("(c p) h -> p c h", p=P))

            if not shared:
                gb_sb = work.tile([P, T], F32, tag="gb")
                g_row = work.tile([1, T], F32, tag="g_row")
                nc.sync.dma_start(g_row[:], gT[e:e + 1, :])
                for half in range(NH):
                    csl = slice(half * 512, (half + 1) * 512)
                    pgb = pm_pool.tile([P, 512], F32, tag="pt")
                    nc.tensor.matmul(pgb[:], ones_row[:], g_row[0:1, csl],
                                     start=True, stop=True)
                    nc.vector.tensor_copy(gb_sb[:, csl], pgb[:])

            z_sb = work.tile([P, LT, T], F32R, tag="z")
            for lt in range(LT):
                lsl = slice(lt * P, (lt + 1) * P)
                for half in range(NH):
                    csl = slice(half * 512, (half + 1) * 512)
                    pht = ph_pool.tile([P, 512], F32, tag="ph")
                    for hc in range(HC):
                        nc.tensor.matmul(pht[:], w1_sb[:, hc, lsl],
                                         xt_r[:, hc, csl],
                                         start=(hc == 0), stop=(hc == HC - 1))
                    bias_ap = (sb1_sb if shared else rb1_sb[:, e, :])[:, lt:lt + 1]
                    nc.scalar.activation(z_sb[:, lt, csl], pht[:], AF.Silu,
                                         bias=bias_ap)
                    if not shared:
                        nc.vector.tensor_mul(z_sb[:, lt, csl], z_sb[:, lt, csl],
                                             gb_sb[:, csl])

            for ht in range(HT):
                hsl = slice(ht * P, (ht + 1) * P)
                for half in range(NH):
                    csl = slice(half * 512, (half + 1) * 512)
                    pot = po_pool.tile([P, 512], F32, tag="po")
                    for lc in range(LT):
                        nc.tensor.matmul(pot[:], w2_sb[:, lc, hsl],
                                         z_sb[:, lc, csl],
                                         start=(lc == 0), stop=(lc == LT - 1))
                    nc.vector.tensor_add(acc[:, ht, csl], acc[:, ht, csl], pot[:])

        for ht in range(HT):
            nc.sync.dma_start(yt[ht * P:(ht + 1) * P, :], acc[:, ht, :])

    nc.compile()
    return nc


def kernel(hidden_states, expert_centroids, expert_biases, sw1, sb1, sw2, sb2,
           rw1, rb1, rw2, rb2, **_ignored):
    args = [np.asarray(a, dtype=np.float32) for a in
            (hidden_states, expert_centroids, expert_biases, sw1, sb1, sw2,
             sb2, rw1, rb1, rw2, rb2)]
    prep = _prep_host(args)
    strategy = os.environ.get("MOE_STRATEGY", "ep2")
    if strategy == "dense":
        return _kernel_dense(prep)
    return _kernel_ep2(prep)


# revision 47
# speedup vs baseline: 1.0054x; 1.0054x over previous
"""DeepSeek-MoE (16 routed experts, top-2, 1 shared expert) on 8 Trainium2 cores.

Default strategy "ep2" (expert-parallel, two launches):
  Stage 1 (data-parallel over tokens): each core takes 1024 of the 8192
  tokens and computes the router (exact fp32), top-2 gates, balance-loss
  stats, plus the shared expert and residual:  y1 = x + shared(x) +
  sum_e g_e*rb2[e] + sb2.  It returns the full gate matrix.
  Host dispatch: tokens are packed per expert from the device-computed
  top-k gates (the "all-to-all" of the sharding hint, done host-side since
  the kernel contract is full-input/full-output).
  Stage 2 (expert-parallel): core c holds routed experts 2c and 2c+1 and
  runs them densely over just their assigned (padded) tokens, applying the
  gate to the silu'd intermediate before the second matmul. Host adds the
  gathered expert outputs into y1.

Big matmuls run as float32r (full-rate fp32 PE mode, ~1e-4 rel err); the
router runs exact fp32 so top-2 selection matches the reference.

Fallback strategy "dense" (MOE_STRATEGY=dense): single launch, each core
computes all 16 experts densely for its token shard.

Shapes hardcoded per the problem spec: hidden_states [4, 2048, 1024],
E=16, L=512, H=1024, top-2.
"""

import os
import sys

for _p in ("/opt/trn_rl_repo", "/root/.axon_site/_ro/trn_rl_repo"):
    if _p not in sys.path:
        sys.path.append(_p)

import numpy as np
from contextlib import ExitStack

import concourse.bass as bass
import concourse.tile as tile
from concourse import bacc, mybir
from concourse.bass_utils import run_bass_kernel_spmd
from concourse.masks import make_identity

F32 = mybir.dt.float32
F32R = mybir.dt.float32r
AF = mybir.ActivationFunctionType
ALU = mybir.AluOpType

P = 128
B, S, H, L, E = 4, 2048, 1024, 512, 16
TOP_K = 2
BALANCE_ALPHA = 0.001
N_CORES = 8
T_FULL = B * S
T = T_FULL // N_CORES  # tokens per core in stage 1 (1024)
HC = H // P  # 8 contraction chunks over H
LT = L // P  # 4 tiles over L
HT = H // P  # 8 tiles over H
NH = T // 512  # moving-dim halves (2)
EPC = E // N_CORES  # routed experts per core in stage 2 (2)


def _col_tiles(c):
    """Split token-column count c into moving-dim tiles of <=512."""
    out, o = [], 0
    while o < c:
        w = min(512, c - o)
        out.append((o, w))
        o += w
    return out


def _router_and_gates(nc, tc, ctx, pools, xt_sb, ct_sb, eb_sb, ones_col,
                      identity, gT, stats_ap):
    """Emit router matmuls (exact fp32), top-2 gating, and balance stats.

    Fills gT [E, T] in SBUF and DMAs stats [E, 2] (mask row-sums, s' row-
    sums) to DRAM.
    """
    small, pm_pool, ps_pool = pools
    aff_T = ctx.enter_context(tc.tile_pool(name="aff", bufs=1))
    aff = aff_T.tile([E, T], F32)
    biased = aff_T.tile([E, T], F32) if eb_sb is not None else aff

    for half in range(NH):
        pa = pm_pool.tile([E, 512], F32, tag="pt")
        for hc in range(HC):
            nc.tensor.matmul(pa[:], ct_sb[:, hc, :],
                             xt_sb[:, hc, half * 512:(half + 1) * 512],
                             start=(hc == 0), stop=(hc == HC - 1))
        nc.scalar.activation(aff[:, half * 512:(half + 1) * 512], pa[:],
                             AF.Sigmoid)
    if eb_sb is not None:
        nc.vector.tensor_scalar_add(biased[:], aff[:], eb_sb[:, 0:1])

    pmask_sum = ps_pool.tile([E, 1], F32)
    psp_sum = ps_pool.tile([E, 1], F32)
    for tt in range(T // P):
        tsl = slice(tt * P, (tt + 1) * P)
        pta = pm_pool.tile([P, E], F32, tag="pt")
        nc.tensor.transpose(pta[:], aff[:, tsl], identity[0:E, 0:E])
        afftok = small.tile([P, E], F32)
        nc.vector.tensor_copy(afftok[:], pta[:])
        if eb_sb is not None:
            ptb = pm_pool.tile([P, E], F32, tag="pt")
            nc.tensor.transpose(ptb[:], biased[:, tsl], identity[0:E, 0:E])
            biastok = small.tile([P, E], F32)
            nc.vector.tensor_copy(biastok[:], ptb[:])
        else:
            biastok = afftok

        mx8 = small.tile([P, 8], F32)
        nc.vector.max(mx8[:], biastok[:])
        mask = small.tile([P, E], F32)
        nc.vector.tensor_scalar(mask[:], biastok[:], mx8[:, 1:2], None,
                                op0=ALU.is_ge)
        sel = small.tile([P, E], F32)
        nc.vector.tensor_mul(sel[:], afftok[:], mask[:])
        den = small.tile([P, 1], F32)
        nc.vector.reduce_sum(den[:], sel[:], axis=mybir.AxisListType.X)
        nc.vector.tensor_scalar_add(den[:], den[:], 1e-8)
        rec = small.tile([P, 1], F32)
        nc.vector.reciprocal(rec[:], den[:])
        g_tok = small.tile([P, E], F32)
        nc.vector.tensor_scalar_mul(g_tok[:], sel[:], rec[:, 0:1])

        den2 = small.tile([P, 1], F32)
        nc.vector.reduce_sum(den2[:], afftok[:], axis=mybir.AxisListType.X)
        nc.vector.tensor_scalar_add(den2[:], den2[:], 1e-8)
        rec2 = small.tile([P, 1], F32)
        nc.vector.reciprocal(rec2[:], den2[:])
        sprime = small.tile([P, E], F32)
        nc.vector.tensor_scalar_mul(sprime[:], afftok[:], rec2[:, 0:1])

        nc.tensor.matmul(pmask_sum[:], mask[:], ones_col[:],
                         start=(tt == 0), stop=(tt == T // P - 1))
        nc.tensor.matmul(psp_sum[:], sprime[:], ones_col[:],
                         start=(tt == 0), stop=(tt == T // P - 1))

        ptg = pm_pool.tile([E, P], F32, tag="pt")
        nc.tensor.transpose(ptg[:], g_tok[:], identity[:])
        nc.vector.tensor_copy(gT[:, tsl], ptg[:])

    stats_sb = small.tile([E, 2], F32)
    nc.vector.tensor_copy(stats_sb[:, 0:1], pmask_sum[:])
    nc.vector.tensor_copy(stats_sb[:, 1:2], psp_sum[:])
    nc.sync.dma_start(stats_ap[:], stats_sb[:])


def _build_stage1(zero_bias=True):
    """Router + gates + stats + shared expert + residual, DP over tokens.

    zero_bias=True omits the expert_biases add and the rb2/sb2 correction
    (all zero in this model); the False variant keeps the exact math.
    """
    nc = bacc.Bacc("TRN2", target_bir_lowering=False, debug=False,
                   num_devices=N_CORES)

    xta = nc.dram_tensor("xta", [P, 3, T], F32, kind="ExternalInput").ap()
    xtb = nc.dram_tensor("xtb", [P, 3, T], F32, kind="ExternalInput").ap()
    xtc = nc.dram_tensor("xtc", [P, 2, T], F32, kind="ExternalInput").ap()
    ct = nc.dram_tensor("ct", [P, HC, E], F32, kind="ExternalInput").ap()
    s1 = nc.dram_tensor("s1", [P, HC, L], F32, kind="ExternalInput").ap()
    s2 = nc.dram_tensor("s2", [P, LT, H], F32, kind="ExternalInput").ap()
    sb1 = nc.dram_tensor("sb1", [P, LT], F32, kind="ExternalInput").ap()
    if not zero_bias:
        eb = nc.dram_tensor("eb", [E, 1], F32, kind="ExternalInput").ap()
        rb2 = nc.dram_tensor("rb2", [E + 1, H], F32, kind="ExternalInput").ap()
    yt = nc.dram_tensor("yt", [H, T], F32, kind="ExternalOutput").ap()
    gt_out = nc.dram_tensor("gt_out", [E, T], F32, kind="ExternalOutput").ap()
    stats = nc.dram_tensor("stats", [E, 2], F32, kind="ExternalOutput").ap()

    with tile.TileContext(nc) as tc, ExitStack() as ctx:
        const = ctx.enter_context(tc.tile_pool(name="const", bufs=1))
        xpool = ctx.enter_context(tc.tile_pool(name="xpool", bufs=1))
        work = ctx.enter_context(tc.tile_pool(name="work", bufs=1))
        small = ctx.enter_context(tc.tile_pool(name="small", bufs=4))
        ph_pool = ctx.enter_context(tc.tile_pool(name="ph", bufs=2, space="PSUM"))
        po_pool = ctx.enter_context(tc.tile_pool(name="po", bufs=2, space="PSUM"))
        pm_pool = ctx.enter_context(tc.tile_pool(name="pm", bufs=2, space="PSUM"))
        ps_pool = ctx.enter_context(tc.tile_pool(name="ps", bufs=1, space="PSUM"))

        identity = const.tile([P, P], F32)
        make_identity(nc, identity)
        ones_col = const.tile([P, 1], F32)
        nc.any.memset(ones_col[:], 1.0)

        # PE warm-up: a few throwaway matmuls during the DMA lead push the
        # HAM clock gate to full rate before the fp32 router matmuls issue
        warm = const.tile([P, 512], F32)
        nc.any.memset(warm[:], 0.0)
        pwarm = pm_pool.tile([P, 512], F32, tag="pt")
        for _ in range(4):
            nc.tensor.matmul(pwarm[:], warm[:, 0:P], warm[:],
                             start=True, stop=True)

        # x (exact fp32) split across both HW DGE queues (SP + ACT);
        # each piece is partition-contiguous in DRAM
        xt_sb = xpool.tile([P, HC, T], F32, tag="x")
        nc.sync.dma_start(xt_sb[:, 0:3, :], xta[:])
        nc.scalar.dma_start(xt_sb[:, 3:6, :], xtb[:])
        nc.sync.dma_start(xt_sb[:, 6:8, :], xtc[:])
        ct_sb = const.tile([P, HC, E], F32)
        nc.scalar.dma_start(ct_sb[:], ct[:])
        sb1_sb = const.tile([P, LT], F32)
        nc.sync.dma_start(sb1_sb[:], sb1[:])

        # shared-expert operands stream on the gpsimd (casting) queue in
        # parallel with the fp32 x load: w1 then per-chunk f32r x so the
        # shared expert's first matmuls start before the router's x lands
        w1_sb = work.tile([P, HC, L], F32R, tag="w1")
        for q in range(4):
            nc.gpsimd.dma_start(w1_sb[:, 2 * q:2 * q + 2, :],
                                s1[:, 2 * q:2 * q + 2, :])
        xt_r = [const.tile([P, T], F32R, name=f"xtr{hc}") for hc in range(HC)]
        for hc in range(HC):
            if hc < 3:
                nc.gpsimd.dma_start(xt_r[hc][:], xta[:, hc, :])
            elif hc < 6:
                nc.gpsimd.dma_start(xt_r[hc][:], xtb[:, hc - 3, :])
            else:
                nc.gpsimd.dma_start(xt_r[hc][:], xtc[:, hc - 6, :])
        w2_sb = work.tile([P, LT, H], F32R, tag="w2")
        for q in range(4):
            nc.gpsimd.dma_start(w2_sb[:, q, :], s2[:, q, :])

        acc = const.tile([P, HT, T], F32)
        halves = [slice(h * 512, (h + 1) * 512) for h in range(NH)]

        # shared-expert first matmul chain emitted BEFORE the router: its
        # f32r operands stream on gpsimd while the exact-fp32 x is still
        # loading, so the PE has work during the router's DMA wait
        z_sb = work.tile([P, LT, T], F32R, tag="z")
        for lt in range(LT):
            lsl = slice(lt * P, (lt + 1) * P)
            phts = [ph_pool.tile([P, 512], F32, tag="ph", name=f"ph{lt}_{h}")
                    for h in range(NH)]
            for hc in range(HC):
                for h, csl in enumerate(halves):
                    nc.tensor.matmul(phts[h][:], w1_sb[:, hc, lsl],
                                     xt_r[hc][:, csl],
                                     start=(hc == 0), stop=(hc == HC - 1))
            for h, csl in enumerate(halves):
                nc.scalar.activation(z_sb[:, lt, csl], phts[h][:], AF.Silu,
                                     bias=sb1_sb[:, lt:lt + 1])

        if zero_bias:
            gT = const.tile([E, T], F32)
            eb_sb = None
        else:
            eb_sb = const.tile([E, 1], F32)
            nc.sync.dma_start(eb_sb[:], eb[:])
            rb2_sb = const.tile([E + 1, H], F32)
            nc.sync.dma_start(rb2_sb[:], rb2[:])
            gTx = const.tile([E + 1, T], F32)
            gT = gTx[0:E, :]
            # row E stays 1.0 (gate for the sb2 bias row)
            nc.any.memset(gTx[:], 1.0)

        _router_and_gates(nc, tc, ctx, (small, pm_pool, ps_pool), xt_sb,
                          ct_sb, eb_sb, ones_col, identity, gT, stats)
        nc.sync.dma_start(gt_out[:], gT[:])

        if not zero_bias:
            # acc = x + sum_e g_e*rb2[e] + sb2   (sb2 rides as gate-1 row E)
            for ht in range(HT):
                hsl = slice(ht * P, (ht + 1) * P)
                for half in range(NH):
                    csl = slice(half * 512, (half + 1) * 512)
                    pc = po_pool.tile([P, 512], F32, tag="po")
                    nc.tensor.matmul(pc[:], rb2_sb[:, hsl], gTx[:, csl],
                                     start=True, stop=True)
                    nc.vector.tensor_add(acc[:, ht, csl], xt_sb[:, ht, csl],
                                         pc[:])

        for ht in range(HT):
            hsl = slice(ht * P, (ht + 1) * P)
            pots = [po_pool.tile([P, 512], F32, tag="po", name=f"po{ht}_{h}")
                    for h in range(NH)]
            for lc in range(LT):
                for h, csl in enumerate(halves):
                    nc.tensor.matmul(pots[h][:], w2_sb[:, lc, hsl],
                                     z_sb[:, lc, csl],
                                     start=(lc == 0), stop=(lc == LT - 1))
            src_ap = xt_sb if zero_bias else acc
            for h, csl in enumerate(halves):
                nc.vector.tensor_add(acc[:, ht, csl], src_ap[:, ht, csl],
                                     pots[h][:])

        # stores split across both HW DGE queues
        for ht in range(HT):
            eng = nc.sync if ht % 2 == 0 else nc.scalar
            eng.dma_start(yt[ht * P:(ht + 1) * P, :], acc[:, ht, :])

    nc.compile()
    return nc


def _build_stage2(C1, C2):
    """Two routed experts per core over their gathered (padded) tokens.

    Slot 0 holds a high-count expert (capacity C1), slot 1 a low-count
    one (C2 <= C1) — the host ranks experts by load so padding is small.
    """
    nc = bacc.Bacc("TRN2", target_bir_lowering=False, debug=False,
                   num_devices=N_CORES)

    caps = (C1, C2)
    xes = [nc.dram_tensor(f"xe{k}", [P, HC, caps[k]], F32,
                          kind="ExternalInput").ap() for k in range(EPC)]
    ges = [nc.dram_tensor(f"ge{k}", [1, caps[k]], F32,
                          kind="ExternalInput").ap() for k in range(EPC)]
    w1p = nc.dram_tensor("w1p", [EPC, P, HC, L], F32, kind="ExternalInput").ap()
    w2p = nc.dram_tensor("w2p", [EPC, P, LT, H], F32, kind="ExternalInput").ap()
    rb1p = nc.dram_tensor("rb1p", [EPC, P, LT], F32, kind="ExternalInput").ap()
    yes = [nc.dram_tensor(f"ye{k}", [H, caps[k]], F32,
                          kind="ExternalOutput").ap() for k in range(EPC)]

    with tile.TileContext(nc) as tc, ExitStack() as ctx:
        const = ctx.enter_context(tc.tile_pool(name="const", bufs=1))
        xep = ctx.enter_context(tc.tile_pool(name="xep", bufs=16))
        xw = ctx.enter_context(tc.tile_pool(name="xw", bufs=2))
        zw = ctx.enter_context(tc.tile_pool(name="zw", bufs=1))
        small = ctx.enter_context(tc.tile_pool(name="small", bufs=4))
        ph_pool = ctx.enter_context(tc.tile_pool(name="ph", bufs=3, space="PSUM"))
        po_pool = ctx.enter_context(tc.tile_pool(name="po", bufs=3, space="PSUM"))
        pm_pool = ctx.enter_context(tc.tile_pool(name="pm", bufs=2, space="PSUM"))

        ones_row = const.tile([1, P], F32)
        nc.any.memset(ones_row[:], 1.0)
        rb1_sb = const.tile([P, EPC, LT], F32)
        nc.sync.dma_start(rb1_sb[:], rb1p.rearrange("k p l -> p k l"))

        warm = const.tile([P, 512], F32)
        nc.any.memset(warm[:], 0.0)
        pwarm = pm_pool.tile([P, 512], F32, tag="pt")
        for _ in range(4):
            nc.tensor.matmul(pwarm[:], warm[:, 0:P], warm[:],
                             start=True, stop=True)

        for k in range(EPC):
            C = caps[k]
            ctiles = _col_tiles(C)
            w1_sb = xw.tile([P, HC, L], F32R, tag="w1")
            for q in range(4):
                nc.gpsimd.dma_start(w1_sb[:, 2 * q:2 * q + 2, :],
                                    w1p[k][:, 2 * q:2 * q + 2, :])
            # per-chunk token tiles: lets expert k+1's loads start while
            # expert k's first matmul chain is still reading its chunks
            xe_r = [xep.tile([P, C], F32R, tag="xec", name=f"xec{k}_{hc}")
                    for hc in range(HC)]
            for hc in range(HC):
                nc.gpsimd.dma_start(xe_r[hc][:], xes[k][:, hc, :])
            w2_sb = xw.tile([P, LT, H], F32R, tag="w2")
            for q in range(4):
                nc.gpsimd.dma_start(w2_sb[:, q, :], w2p[k][:, q, :])

            g_row = xw.tile([1, C], F32, tag="g_row")
            nc.sync.dma_start(g_row[:], ges[k])
            gb_sb = xw.tile([P, C], F32, tag="gb")
            for co, cw in ctiles:
                pgb = pm_pool.tile([P, 512], F32, tag="pt")
                nc.tensor.matmul(pgb[:, :cw], ones_row[:], g_row[0:1, co:co + cw],
                                 start=True, stop=True)
                nc.vector.tensor_copy(gb_sb[:, co:co + cw], pgb[:, :cw])

            # group col tiles in pairs so each weight load serves 2 matmuls
            ctpairs = [ctiles[i:i + 2] for i in range(0, len(ctiles), 2)]
            z_sb = zw.tile([P, LT, C], F32R, tag="z")
            for lt in range(LT):
                lsl = slice(lt * P, (lt + 1) * P)
                for pi, pair in enumerate(ctpairs):
                    phts = [ph_pool.tile([P, 512], F32, tag="ph",
                                         name=f"ph{k}_{lt}_{pi}_{j}")
                            for j in range(len(pair))]
                    for hc in range(HC):
                        for j, (co, cw) in enumerate(pair):
                            nc.tensor.matmul(
                                phts[j][:, :cw], w1_sb[:, hc, lsl],
                                xe_r[hc][:, co:co + cw],
                                start=(hc == 0), stop=(hc == HC - 1))
                    for j, (co, cw) in enumerate(pair):
                        csl = slice(co, co + cw)
                        nc.scalar.activation(z_sb[:, lt, csl], phts[j][:, :cw],
                                             AF.Silu,
                                             bias=rb1_sb[:, k, lt:lt + 1])
                        nc.vector.tensor_mul(z_sb[:, lt, csl],
                                             z_sb[:, lt, csl], gb_sb[:, csl])

            for ht in range(HT):
                hsl = slice(ht * P, (ht + 1) * P)
                for pi, pair in enumerate(ctpairs):
                    pots = [po_pool.tile([P, 512], F32, tag="po",
                                         name=f"po{k}_{ht}_{pi}_{j}")
                            for j in range(len(pair))]
                    for lc in range(LT):
                        for j, (co, cw) in enumerate(pair):
                            nc.tensor.matmul(
                                pots[j][:, :cw], w2_sb[:, lc, hsl],
                                z_sb[:, lc, co:co + cw],
                                start=(lc == 0), stop=(lc == LT - 1))
                    for j, (co, cw) in enumerate(pair):
                        csl = slice(co, co + cw)
                        stg = small.tile([P, 512], F32, tag="stg")
                        nc.vector.tensor_copy(stg[:, :cw], pots[j][:, :cw])
                        eng = nc.sync if (ht + pi + j) % 2 == 0 else nc.scalar
                        eng.dma_start(yes[k][ht * P:(ht + 1) * P, csl],
                                      stg[:, :cw])

    nc.compile()
    return nc


_NC_CACHE = {}


def _get(name, builder, *args):
    key = (name,) + args
    if key not in _NC_CACHE:
        _NC_CACHE[key] = builder(*args)
    return _NC_CACHE[key]


def _prep_host(inputs):
    (hidden_states, expert_centroids, expert_biases, sw1, sb1, sw2, sb2,
     rw1, rb1, rw2, rb2) = inputs
    flat = hidden_states.reshape(T_FULL, H)
    prep = {
        "flat": flat,
        "ct": np.ascontiguousarray(expert_centroids.T),
        "eb": np.ascontiguousarray(expert_biases.reshape(E, 1)),
        "s1": np.ascontiguousarray(sw1.sum(axis=0)),
        "s2": np.ascontiguousarray(sw2.sum(axis=0)),
        "sb1": np.ascontiguousarray(sb1.sum(axis=0).reshape(LT, P).T),
        "sb2": np.ascontiguousarray(sb2.sum(axis=0).reshape(HT, P).T),
        "rb1": np.ascontiguousarray(rb1.reshape(E, LT, P).transpose(0, 2, 1)),
        "rb2": rb2, "rw1": rw1, "rw2": rw2,
        "rb2x": np.ascontiguousarray(
            np.concatenate([rb2, sb2.sum(axis=0)[None]], axis=0)),
    }
    return prep


def _aux_from_stats(mask_sum, sp_sum):
    f_i = mask_sum * (E / (TOP_K * S)) / B
    p_i = sp_sum / T_FULL
    return np.float32(BALANCE_ALPHA * float((f_i * p_i).sum()))


def _perm_h(w):
    """[H, N] -> partition-major [P, HC, N] (contiguous per partition)."""
    return np.ascontiguousarray(w.reshape(-1, P, w.shape[-1]).transpose(1, 0, 2))


def _kernel_ep2(prep):
    zero_bias = (not prep["eb"].any()) and (not prep["rb2x"].any())
    nc1 = _get("s1", _build_stage1, zero_bias)
    flatT_perm = _perm_h(np.ascontiguousarray(prep["flat"].T))  # [P, HC, TF]
    ct_p = _perm_h(prep["ct"])
    s1_p = _perm_h(prep["s1"])
    s2_p = _perm_h(prep["s2"])
    in1 = []
    for c in range(N_CORES):
        xt_c = flatT_perm[:, :, c * T:(c + 1) * T]
        m = {"xta": np.ascontiguousarray(xt_c[:, 0:3]),
             "xtb": np.ascontiguousarray(xt_c[:, 3:6]),
             "xtc": np.ascontiguousarray(xt_c[:, 6:8]),
             "ct": ct_p, "s1": s1_p, "s2": s2_p, "sb1": prep["sb1"]}
        if not zero_bias:
            m["eb"] = prep["eb"]
            m["rb2"] = prep["rb2x"]
        in1.append(m)
    res1 = run_bass_kernel_spmd(nc1, in1, core_ids=list(range(N_CORES)))

    mask_sum = np.zeros(E, dtype=np.float64)
    sp_sum = np.zeros(E, dtype=np.float64)
    gT_full = np.empty((E, T_FULL), dtype=np.float32)
    out = np.empty((T_FULL, H), dtype=np.float32)
    for c in range(N_CORES):
        r = res1.results[c]
        out[c * T:(c + 1) * T] = r["yt"].T
        gT_full[:, c * T:(c + 1) * T] = r["gt_out"]
        mask_sum += r["stats"][:, 0]
        sp_sum += r["stats"][:, 1]
    aux = _aux_from_stats(mask_sum, sp_sum)

    # token dispatch on the device-computed top-k gates; rank experts by
    # load so the low-count slot can use a smaller capacity
    idx = [np.nonzero(gT_full[e] > 0)[0] for e in range(E)]
    order = sorted(range(E), key=lambda e: -len(idx[e]))
    # core c: slot 0 <- rank c (heavy), slot 1 <- rank 15-c (light)
    assign = [(order[c], order[E - 1 - c]) for c in range(N_CORES)]
    rup = lambda n: max(512, -(-n // 256) * 256)
    C1 = rup(max(len(idx[e]) for e, _ in assign))
    C2 = rup(max(len(idx[e]) for _, e in assign))

    w1_perm = np.ascontiguousarray(
        prep["rw1"].reshape(E, HC, P, L).transpose(0, 2, 1, 3))
    w2_perm = np.ascontiguousarray(
        prep["rw2"].reshape(E, LT, P, H).transpose(0, 2, 1, 3))

    nc2 = _get("s2", _build_stage2, C1, C2)
    caps = (C1, C2)
    in2 = []
    for c in range(N_CORES):
        m = {}
        for k in range(EPC):
            e = assign[c][k]
            n = len(idx[e])
            xe = np.zeros((P, HC, caps[k]), dtype=np.float32)
            gevals = np.zeros((1, caps[k]), dtype=np.float32)
            xe[:, :, :n] = flatT_perm[:, :, idx[e]]
            gevals[0, :n] = gT_full[e, idx[e]]
            m[f"xe{k}"] = xe
            m[f"ge{k}"] = gevals
        es = list(assign[c])
        m["w1p"] = np.ascontiguousarray(w1_perm[es])
        m["w2p"] = np.ascontiguousarray(w2_perm[es])
        m["rb1p"] = np.ascontiguousarray(prep["rb1"][es])
        in2.append(m)
    res2 = run_bass_kernel_spmd(nc2, in2, core_ids=list(range(N_CORES)))

    for c in range(N_CORES):
        for k in range(EPC):
            e = assign[c][k]
            n = len(idx[e])
            out[idx[e]] += res2.results[c][f"ye{k}"][:, :n].T
    return out.reshape(B, S, H), aux


def _kernel_dense(prep):
    nc = _get("dense", _build_dense)
    in_maps = []
    for c in range(N_CORES):
        xt_c = np.ascontiguousarray(prep["flat"][c * T:(c + 1) * T].T)
        in_maps.append({
            "xt": xt_c, "ct": prep["ct"], "eb": prep["eb"],
            "w1": prep["rw1"], "w2": prep["rw2"], "s1": prep["s1"],
            "s2": prep["s2"], "rb1": prep["rb1"], "rb2": prep["rb2"],
            "sb1": prep["sb1"], "sb2": prep["sb2"],
        })
    res = run_bass_kernel_spmd(nc, in_maps, core_ids=list(range(N_CORES)))

    out = np.empty((T_FULL, H), dtype=np.float32)
    mask_sum = np.zeros(E, dtype=np.float64)
    sp_sum = np.zeros(E, dtype=np.float64)
    for c in range(N_CORES):
        out[c * T:(c + 1) * T] = res.results[c]["yt"].T
        st = res.results[c]["stats"]
        mask_sum += st[:, 0]
        sp_sum += st[:, 1]
    return out.reshape(B, S, H), _aux_from_stats(mask_sum, sp_sum)


def _build_dense():
    """Single-launch fallback: every core runs all experts on its shard."""
    nc = bacc.Bacc("TRN2", target_bir_lowering=False, debug=False,
                   num_devices=N_CORES)

    xt = nc.dram_tensor("xt", [H, T], F32, kind="ExternalInput").ap()
    ct = nc.dram_tensor("ct", [H, E], F32, kind="ExternalInput").ap()
    eb = nc.dram_tensor("eb", [E, 1], F32, kind="ExternalInput").ap()
    w1 = nc.dram_tensor("w1", [E, H, L], F32, kind="ExternalInput").ap()
    w2 = nc.dram_tensor("w2", [E, L, H], F32, kind="ExternalInput").ap()
    s1 = nc.dram_tensor("s1", [H, L], F32, kind="ExternalInput").ap()
    s2 = nc.dram_tensor("s2", [L, H], F32, kind="ExternalInput").ap()
    rb1 = nc.dram_tensor("rb1", [E, P, LT], F32, kind="ExternalInput").ap()
    rb2 = nc.dram_tensor("rb2", [E, H], F32, kind="ExternalInput").ap()
    sb1 = nc.dram_tensor("sb1", [P, LT], F32, kind="ExternalInput").ap()
    sb2 = nc.dram_tensor("sb2", [P, HT], F32, kind="ExternalInput").ap()
    yt = nc.dram_tensor("yt", [H, T], F32, kind="ExternalOutput").ap()
    stats = nc.dram_tensor("stats", [E, 2], F32, kind="ExternalOutput").ap()

    with tile.TileContext(nc) as tc, ExitStack() as ctx:
        const = ctx.enter_context(tc.tile_pool(name="const", bufs=1))
        xpool = ctx.enter_context(tc.tile_pool(name="xpool", bufs=1))
        work = ctx.enter_context(tc.tile_pool(name="work", bufs=2))
        small = ctx.enter_context(tc.tile_pool(name="small", bufs=4))
        ph_pool = ctx.enter_context(tc.tile_pool(name="ph", bufs=2, space="PSUM"))
        po_pool = ctx.enter_context(tc.tile_pool(name="po", bufs=2, space="PSUM"))
        pm_pool = ctx.enter_context(tc.tile_pool(name="pm", bufs=2, space="PSUM"))
        ps_pool = ctx.enter_context(tc.tile_pool(name="ps", bufs=1, space="PSUM"))

        identity = const.tile([P, P], F32)
        make_identity(nc, identity)
        ones_col = const.tile([P, 1], F32)
        nc.any.memset(ones_col[:], 1.0)
        ones_row = const.tile([1, P], F32)
        nc.any.memset(ones_row[:], 1.0)

        xt_sb = xpool.tile([P, HC, T], F32, tag="x")
        nc.sync.dma_start(xt_sb[:], xt.rearrange("(c p) t -> p c t", p=P))
        ct_sb = const.tile([P, HC, E], F32)
        nc.sync.dma_start(ct_sb[:], ct.rearrange("(c p) e -> p c e", p=P))
        eb_sb = const.tile([E, 1], F32)
        nc.sync.dma_start(eb_sb[:], eb[:])
        rb2_sb = const.tile([E, H], F32)
        nc.sync.dma_start(rb2_sb[:], rb2[:])
        rb1_sb = const.tile([P, E, LT], F32)
        nc.sync.dma_start(rb1_sb[:], rb1.rearrange("e p l -> p e l"))
        sb1_sb = const.tile([P, LT], F32)
        nc.sync.dma_start(sb1_sb[:], sb1[:])
        sb2_sb = const.tile([P, HT], F32)
        nc.sync.dma_start(sb2_sb[:], sb2[:])

        acc = const.tile([P, HT, T], F32)
        gT = const.tile([E, T], F32)

        _router_and_gates(nc, tc, ctx, (small, pm_pool, ps_pool), xt_sb,
                          ct_sb, eb_sb, ones_col, identity, gT, stats)

        for ht in range(HT):
            hsl = slice(ht * P, (ht + 1) * P)
            for half in range(NH):
                csl = slice(half * 512, (half + 1) * 512)
                pc = po_pool.tile([P, 512], F32, tag="po")
                nc.tensor.matmul(pc[:], rb2_sb[:, hsl], gT[:, csl],
                                 start=True, stop=True)
                nc.vector.tensor_add(acc[:, ht, csl], xt_sb[:, ht, csl], pc[:])
                nc.vector.tensor_scalar_add(acc[:, ht, csl], acc[:, ht, csl],
                                            sb2_sb[:, ht:ht + 1])

        xt_r = xpool.tile([P, HC, T], F32R, tag="x")
        nc.gpsimd.dma_start(xt_r[:], xt.rearrange("(c p) t -> p c t", p=P))

        for e in range(E + 1):
            shared = (e == E)
            w1_sb = work.tile([P, HC, L], F32R, tag="w1")
            w2_sb = work.tile([P, LT, H], F32R, tag="w2")
            if shared:
                nc.gpsimd.dma_start(w1_sb[:], s1.rearrange("(c p) l -> p c l", p=P))
                nc.gpsimd.dma_start(w2_sb[:], s2.rearrange("(c p) h -> p c h", p=P))
            else:
                nc.gpsimd.dma_start(w1_sb[:], w1[e].rearrange("(c p) l -> p c l", p=P))
                nc.gpsimd.dma_start(w2_sb[:], w2[e].rearrange("(c p) h -> p c h", p=P))

            if not shared:
                gb_sb = work.tile([P, T], F32, tag="gb")
                g_row = work.tile([1, T], F32, tag="g_row")
                nc.sync.dma_start(g_row[:], gT[e:e + 1, :])
                for half in range(NH):
                    csl = slice(half * 512, (half + 1) * 512)
                    pgb = pm_pool.tile([P, 512], F32, tag="pt")
                    nc.tensor.matmul(pgb[:], ones_row[:], g_row[0:1, csl],
                                     start=True, stop=True)
                    nc.vector.tensor_copy(gb_sb[:, csl], pgb[:])

            z_sb = work.tile([P, LT, T], F32R, tag="z")
            for lt in range(LT):
                lsl = slice(lt * P, (lt + 1) * P)
                for half in range(NH):
                    csl = slice(half * 512, (half + 1) * 512)
                    pht = ph_pool.tile([P, 512], F32, tag="ph")
                    for hc in range(HC):
                        nc.tensor.matmul(pht[:], w1_sb[:, hc, lsl],
                                         xt_r[:, hc, csl],
                                         start=(hc == 0), stop=(hc == HC - 1))
                    bias_ap = (sb1_sb if shared else rb1_sb[:, e, :])[:, lt:lt + 1]
                    nc.scalar.activation(z_sb[:, lt, csl], pht[:], AF.Silu,
                                         bias=bias_ap)
                    if not shared:
                        nc.vector.tensor_mul(z_sb[:, lt, csl], z_sb[:, lt, csl],
                                             gb_sb[:, csl])

            for ht in range(HT):
                hsl = slice(ht * P, (ht + 1) * P)
                for half in range(NH):
                    csl = slice(half * 512, (half + 1) * 512)
                    pot = po_pool.tile([P, 512], F32, tag="po")
                    for lc in range(LT):
                        nc.tensor.matmul(pot[:], w2_sb[:, lc, hsl],
                                         z_sb[:, lc, csl],
                                         start=(lc == 0), stop=(lc == LT - 1))
                    nc.vector.tensor_add(acc[:, ht, csl], acc[:, ht, csl], pot[:])

        for ht in range(HT):
            nc.sync.dma_start(yt[ht * P:(ht + 1) * P, :], acc[:, ht, :])

    nc.compile()
    return nc


def kernel(hidden_states, expert_centroids, expert_biases, sw1, sb1, sw2, sb2,
           rw1, rb1, rw2, rb2, **_ignored):
    args = [np.asarray(a, dtype=np.float32) for a in
            (hidden_states, expert_centroids, expert_biases, sw1, sb1, sw2,
             sb2, rw1, rb1, rw2, rb2)]
    prep = _prep_host(args)
    strategy = os.environ.get("MOE_STRATEGY", "ep2")
    if strategy == "dense":
        return _kernel_dense(prep)
    return _kernel_ep2(prep)


# revision 48
# speedup vs baseline: 1.0318x; 1.0263x over previous
"""DeepSeek-MoE (16 routed experts, top-2, 1 shared expert) on 8 Trainium2 cores.

Default strategy "ep2" (expert-parallel, two launches):
  Stage 1 (data-parallel over tokens): each core takes 1024 of the 8192
  tokens and computes the router (exact fp32), top-2 gates, balance-loss
  stats, plus the shared expert and residual:  y1 = x + shared(x) +
  sum_e g_e*rb2[e] + sb2.  It returns the full gate matrix.
  Host dispatch: tokens are packed per expert from the device-computed
  top-k gates (the "all-to-all" of the sharding hint, done host-side since
  the kernel contract is full-input/full-output).
  Stage 2 (expert-parallel): core c holds routed experts 2c and 2c+1 and
  runs them densely over just their assigned (padded) tokens, applying the
  gate to the silu'd intermediate before the second matmul. Host adds the
  gathered expert outputs into y1.

Big matmuls run as float32r (full-rate fp32 PE mode, ~1e-4 rel err); the
router runs exact fp32 so top-2 selection matches the reference.

Fallback strategy "dense" (MOE_STRATEGY=dense): single launch, each core
computes all 16 experts densely for its token shard.

Shapes hardcoded per the problem spec: hidden_states [4, 2048, 1024],
E=16, L=512, H=1024, top-2.
"""

import os
import sys

for _p in ("/opt/trn_rl_repo", "/root/.axon_site/_ro/trn_rl_repo"):
    if _p not in sys.path:
        sys.path.append(_p)

import numpy as np
from contextlib import ExitStack

import concourse.bass as bass
import concourse.tile as tile
from concourse import bacc, mybir
from concourse.bass_utils import run_bass_kernel_spmd
from concourse.masks import make_identity

F32 = mybir.dt.float32
F32R = mybir.dt.float32r
AF = mybir.ActivationFunctionType
ALU = mybir.AluOpType

P = 128
B, S, H, L, E = 4, 2048, 1024, 512, 16
TOP_K = 2
BALANCE_ALPHA = 0.001
N_CORES = 8
T_FULL = B * S
T = T_FULL // N_CORES  # tokens per core in stage 1 (1024)
HC = H // P  # 8 contraction chunks over H
LT = L // P  # 4 tiles over L
HT = H // P  # 8 tiles over H
NH = T // 512  # moving-dim halves (2)
EPC = E // N_CORES  # routed experts per core in stage 2 (2)


def _col_tiles(c):
    """Split token-column count c into moving-dim tiles of <=512."""
    out, o = [], 0
    while o < c:
        w = min(512, c - o)
        out.append((o, w))
        o += w
    return out


def _router_and_gates(nc, tc, ctx, pools, xt_sb, ct_sb, eb_sb, ones_col,
                      identity, gT, stats_ap):
    """Emit router matmuls (exact fp32), top-2 gating, and balance stats.

    Fills gT [E, T] in SBUF and DMAs stats [E, 2] (mask row-sums, s' row-
    sums) to DRAM.
    """
    small, pm_pool, ps_pool = pools
    aff_T = ctx.enter_context(tc.tile_pool(name="aff", bufs=1))
    aff = aff_T.tile([E, T], F32)
    biased = aff_T.tile([E, T], F32) if eb_sb is not None else aff

    for half in range(NH):
        pa = pm_pool.tile([E, 512], F32, tag="pt")
        for hc in range(HC):
            nc.tensor.matmul(pa[:], ct_sb[:, hc, :],
                             xt_sb[:, hc, half * 512:(half + 1) * 512],
                             start=(hc == 0), stop=(hc == HC - 1))
        nc.scalar.activation(aff[:, half * 512:(half + 1) * 512], pa[:],
                             AF.Sigmoid)
    if eb_sb is not None:
        nc.vector.tensor_scalar_add(biased[:], aff[:], eb_sb[:, 0:1])

    pmask_sum = ps_pool.tile([E, 1], F32)
    psp_sum = ps_pool.tile([E, 1], F32)
    for tt in range(T // P):
        tsl = slice(tt * P, (tt + 1) * P)
        pta = pm_pool.tile([P, E], F32, tag="pt")
        nc.tensor.transpose(pta[:], aff[:, tsl], identity[0:E, 0:E])
        afftok = small.tile([P, E], F32)
        nc.vector.tensor_copy(afftok[:], pta[:])
        if eb_sb is not None:
            ptb = pm_pool.tile([P, E], F32, tag="pt")
            nc.tensor.transpose(ptb[:], biased[:, tsl], identity[0:E, 0:E])
            biastok = small.tile([P, E], F32)
            nc.vector.tensor_copy(biastok[:], ptb[:])
        else:
            biastok = afftok

        mx8 = small.tile([P, 8], F32)
        nc.vector.max(mx8[:], biastok[:])
        mask = small.tile([P, E], F32)
        nc.vector.tensor_scalar(mask[:], biastok[:], mx8[:, 1:2], None,
                                op0=ALU.is_ge)
        sel = small.tile([P, E], F32)
        nc.vector.tensor_mul(sel[:], afftok[:], mask[:])
        den = small.tile([P, 1], F32)
        nc.vector.reduce_sum(den[:], sel[:], axis=mybir.AxisListType.X)
        nc.vector.tensor_scalar_add(den[:], den[:], 1e-8)
        rec = small.tile([P, 1], F32)
        nc.vector.reciprocal(rec[:], den[:])
        g_tok = small.tile([P, E], F32)
        nc.vector.tensor_scalar_mul(g_tok[:], sel[:], rec[:, 0:1])

        den2 = small.tile([P, 1], F32)
        nc.vector.reduce_sum(den2[:], afftok[:], axis=mybir.AxisListType.X)
        nc.vector.tensor_scalar_add(den2[:], den2[:], 1e-8)
        rec2 = small.tile([P, 1], F32)
        nc.vector.reciprocal(rec2[:], den2[:])
        sprime = small.tile([P, E], F32)
        nc.vector.tensor_scalar_mul(sprime[:], afftok[:], rec2[:, 0:1])

        nc.tensor.matmul(pmask_sum[:], mask[:], ones_col[:],
                         start=(tt == 0), stop=(tt == T // P - 1))
        nc.tensor.matmul(psp_sum[:], sprime[:], ones_col[:],
                         start=(tt == 0), stop=(tt == T // P - 1))

        ptg = pm_pool.tile([E, P], F32, tag="pt")
        nc.tensor.transpose(ptg[:], g_tok[:], identity[:])
        nc.vector.tensor_copy(gT[:, tsl], ptg[:])

    stats_sb = small.tile([E, 2], F32)
    nc.vector.tensor_copy(stats_sb[:, 0:1], pmask_sum[:])
    nc.vector.tensor_copy(stats_sb[:, 1:2], psp_sum[:])
    nc.sync.dma_start(stats_ap[:], stats_sb[:])


def _build_stage1(zero_bias=True):
    """Router + gates + stats + shared expert + residual, DP over tokens.

    zero_bias=True omits the expert_biases add and the rb2/sb2 correction
    (all zero in this model); the False variant keeps the exact math.
    """
    nc = bacc.Bacc("TRN2", target_bir_lowering=False, debug=False,
                   num_devices=N_CORES)

    xta = nc.dram_tensor("xta", [P, 3, T], F32, kind="ExternalInput").ap()
    xtb = nc.dram_tensor("xtb", [P, 3, T], F32, kind="ExternalInput").ap()
    xtc = nc.dram_tensor("xtc", [P, 2, T], F32, kind="ExternalInput").ap()
    ct = nc.dram_tensor("ct", [P, HC, E], F32, kind="ExternalInput").ap()
    s1 = nc.dram_tensor("s1", [P, HC, L], F32, kind="ExternalInput").ap()
    s2 = nc.dram_tensor("s2", [P, LT, H], F32, kind="ExternalInput").ap()
    sb1 = nc.dram_tensor("sb1", [P, LT], F32, kind="ExternalInput").ap()
    if not zero_bias:
        eb = nc.dram_tensor("eb", [E, 1], F32, kind="ExternalInput").ap()
        rb2 = nc.dram_tensor("rb2", [E + 1, H], F32, kind="ExternalInput").ap()
    yt = nc.dram_tensor("yt", [H, T], F32, kind="ExternalOutput").ap()
    gt_out = nc.dram_tensor("gt_out", [E, T], F32, kind="ExternalOutput").ap()
    stats = nc.dram_tensor("stats", [E, 2], F32, kind="ExternalOutput").ap()

    with tile.TileContext(nc) as tc, ExitStack() as ctx:
        const = ctx.enter_context(tc.tile_pool(name="const", bufs=1))
        xpool = ctx.enter_context(tc.tile_pool(name="xpool", bufs=1))
        work = ctx.enter_context(tc.tile_pool(name="work", bufs=1))
        small = ctx.enter_context(tc.tile_pool(name="small", bufs=4))
        ph_pool = ctx.enter_context(tc.tile_pool(name="ph", bufs=2, space="PSUM"))
        po_pool = ctx.enter_context(tc.tile_pool(name="po", bufs=2, space="PSUM"))
        pm_pool = ctx.enter_context(tc.tile_pool(name="pm", bufs=2, space="PSUM"))
        ps_pool = ctx.enter_context(tc.tile_pool(name="ps", bufs=1, space="PSUM"))

        identity = const.tile([P, P], F32)
        make_identity(nc, identity)
        ones_col = const.tile([P, 1], F32)
        nc.any.memset(ones_col[:], 1.0)

        # PE warm-up: a few throwaway matmuls during the DMA lead push the
        # HAM clock gate to full rate before the fp32 router matmuls issue
        warm = const.tile([P, 512], F32)
        nc.any.memset(warm[:], 0.0)
        pwarm = pm_pool.tile([P, 512], F32, tag="pt")
        for _ in range(4):
            nc.tensor.matmul(pwarm[:], warm[:, 0:P], warm[:],
                             start=True, stop=True)

        # x (exact fp32) split across both HW DGE queues (SP + ACT);
        # each piece is partition-contiguous in DRAM
        xt_sb = xpool.tile([P, HC, T], F32, tag="x")
        nc.sync.dma_start(xt_sb[:, 0:3, :], xta[:])
        nc.scalar.dma_start(xt_sb[:, 3:6, :], xtb[:])
        nc.sync.dma_start(xt_sb[:, 6:8, :], xtc[:])
        ct_sb = const.tile([P, HC, E], F32)
        nc.scalar.dma_start(ct_sb[:], ct[:])
        sb1_sb = const.tile([P, LT], F32)
        nc.sync.dma_start(sb1_sb[:], sb1[:])

        # shared-expert operands stream on the gpsimd (casting) queue in
        # parallel with the fp32 x load: w1 then per-chunk f32r x so the
        # shared expert's first matmuls start before the router's x lands
        w1_sb = work.tile([P, HC, L], F32R, tag="w1")
        for q in range(4):
            nc.gpsimd.dma_start(w1_sb[:, 2 * q:2 * q + 2, :],
                                s1[:, 2 * q:2 * q + 2, :])
        xt_r = [const.tile([P, T], F32R, name=f"xtr{hc}") for hc in range(HC)]
        for hc in range(HC):
            if hc < 3:
                nc.gpsimd.dma_start(xt_r[hc][:], xta[:, hc, :])
            elif hc < 6:
                nc.gpsimd.dma_start(xt_r[hc][:], xtb[:, hc - 3, :])
            else:
                nc.gpsimd.dma_start(xt_r[hc][:], xtc[:, hc - 6, :])
        w2_sb = work.tile([P, LT, H], F32R, tag="w2")
        for q in range(4):
            nc.gpsimd.dma_start(w2_sb[:, q, :], s2[:, q, :])

        acc = const.tile([P, HT, T], F32)
        halves = [slice(h * 512, (h + 1) * 512) for h in range(NH)]

        # shared-expert first matmul chain emitted BEFORE the router: its
        # f32r operands stream on gpsimd while the exact-fp32 x is still
        # loading, so the PE has work during the router's DMA wait
        z_sb = work.tile([P, LT, T], F32R, tag="z")
        for lt in range(LT):
            lsl = slice(lt * P, (lt + 1) * P)
            phts = [ph_pool.tile([P, 512], F32, tag="ph", name=f"ph{lt}_{h}")
                    for h in range(NH)]
            for hc in range(HC):
                for h, csl in enumerate(halves):
                    nc.tensor.matmul(phts[h][:], w1_sb[:, hc, lsl],
                                     xt_r[hc][:, csl],
                                     start=(hc == 0), stop=(hc == HC - 1))
            for h, csl in enumerate(halves):
                nc.scalar.activation(z_sb[:, lt, csl], phts[h][:], AF.Silu,
                                     bias=sb1_sb[:, lt:lt + 1])

        if zero_bias:
            gT = const.tile([E, T], F32)
            eb_sb = None
        else:
            eb_sb = const.tile([E, 1], F32)
            nc.sync.dma_start(eb_sb[:], eb[:])
            rb2_sb = const.tile([E + 1, H], F32)
            nc.sync.dma_start(rb2_sb[:], rb2[:])
            gTx = const.tile([E + 1, T], F32)
            gT = gTx[0:E, :]
            # row E stays 1.0 (gate for the sb2 bias row)
            nc.any.memset(gTx[:], 1.0)

        _router_and_gates(nc, tc, ctx, (small, pm_pool, ps_pool), xt_sb,
                          ct_sb, eb_sb, ones_col, identity, gT, stats)
        nc.sync.dma_start(gt_out[:], gT[:])

        if not zero_bias:
            # acc = x + sum_e g_e*rb2[e] + sb2   (sb2 rides as gate-1 row E)
            for ht in range(HT):
                hsl = slice(ht * P, (ht + 1) * P)
                for half in range(NH):
                    csl = slice(half * 512, (half + 1) * 512)
                    pc = po_pool.tile([P, 512], F32, tag="po")
                    nc.tensor.matmul(pc[:], rb2_sb[:, hsl], gTx[:, csl],
                                     start=True, stop=True)
                    nc.vector.tensor_add(acc[:, ht, csl], xt_sb[:, ht, csl],
                                         pc[:])

        for ht in range(HT):
            hsl = slice(ht * P, (ht + 1) * P)
            pots = [po_pool.tile([P, 512], F32, tag="po", name=f"po{ht}_{h}")
                    for h in range(NH)]
            for lc in range(LT):
                for h, csl in enumerate(halves):
                    nc.tensor.matmul(pots[h][:], w2_sb[:, lc, hsl],
                                     z_sb[:, lc, csl],
                                     start=(lc == 0), stop=(lc == LT - 1))
            src_ap = xt_sb if zero_bias else acc
            for h, csl in enumerate(halves):
                nc.vector.tensor_add(acc[:, ht, csl], src_ap[:, ht, csl],
                                     pots[h][:])

        # stores split across both HW DGE queues
        for ht in range(HT):
            eng = nc.sync if ht % 2 == 0 else nc.scalar
            eng.dma_start(yt[ht * P:(ht + 1) * P, :], acc[:, ht, :])

    nc.compile()
    return nc


def _build_stage2(C1, C2):
    """Two routed experts per core over their gathered (padded) tokens.

    Slot 0 holds a high-count expert (capacity C1), slot 1 a low-count
    one (C2 <= C1) — the host ranks experts by load so padding is small.
    """
    nc = bacc.Bacc("TRN2", target_bir_lowering=False, debug=False,
                   num_devices=N_CORES)

    caps = (C1, C2)
    xes = [nc.dram_tensor(f"xe{k}", [P, HC, caps[k]], F32,
                          kind="ExternalInput").ap() for k in range(EPC)]
    ges = [nc.dram_tensor(f"ge{k}", [1, caps[k]], F32,
                          kind="ExternalInput").ap() for k in range(EPC)]
    w1p = nc.dram_tensor("w1p", [EPC, P, HC, L], F32, kind="ExternalInput").ap()
    w2p = nc.dram_tensor("w2p", [EPC, P, LT, H], F32, kind="ExternalInput").ap()
    rb1p = nc.dram_tensor("rb1p", [EPC, P, LT], F32, kind="ExternalInput").ap()
    yes = [nc.dram_tensor(f"ye{k}", [H, caps[k]], F32,
                          kind="ExternalOutput").ap() for k in range(EPC)]

    with tile.TileContext(nc) as tc, ExitStack() as ctx:
        const = ctx.enter_context(tc.tile_pool(name="const", bufs=1))
        xep = ctx.enter_context(tc.tile_pool(name="xep", bufs=16))
        xw = ctx.enter_context(tc.tile_pool(name="xw", bufs=2))
        zw = ctx.enter_context(tc.tile_pool(name="zw", bufs=1))
        small = ctx.enter_context(tc.tile_pool(name="small", bufs=4))
        ph_pool = ctx.enter_context(tc.tile_pool(name="ph", bufs=3, space="PSUM"))
        po_pool = ctx.enter_context(tc.tile_pool(name="po", bufs=4, space="PSUM"))
        pm_pool = ctx.enter_context(tc.tile_pool(name="pm", bufs=1, space="PSUM"))

        ones_row = const.tile([1, P], F32)
        nc.any.memset(ones_row[:], 1.0)
        rb1_sb = const.tile([P, EPC, LT], F32)
        nc.sync.dma_start(rb1_sb[:], rb1p.rearrange("k p l -> p k l"))

        warm = const.tile([P, 512], F32)
        nc.any.memset(warm[:], 0.0)
        pwarm = pm_pool.tile([P, 512], F32, tag="pt")
        for _ in range(4):
            nc.tensor.matmul(pwarm[:], warm[:, 0:P], warm[:],
                             start=True, stop=True)

        for k in range(EPC):
            C = caps[k]
            ctiles = _col_tiles(C)
            w1_sb = xw.tile([P, HC, L], F32R, tag="w1")
            for q in range(4):
                nc.gpsimd.dma_start(w1_sb[:, 2 * q:2 * q + 2, :],
                                    w1p[k][:, 2 * q:2 * q + 2, :])
            # per-chunk token tiles: lets expert k+1's loads start while
            # expert k's first matmul chain is still reading its chunks
            xe_r = [xep.tile([P, C], F32R, tag="xec", name=f"xec{k}_{hc}")
                    for hc in range(HC)]
            for hc in range(HC):
                nc.gpsimd.dma_start(xe_r[hc][:], xes[k][:, hc, :])
            w2_sb = xw.tile([P, LT, H], F32R, tag="w2")
            for q in range(4):
                nc.gpsimd.dma_start(w2_sb[:, q, :], w2p[k][:, q, :])

            g_row = xw.tile([1, C], F32, tag="g_row")
            nc.sync.dma_start(g_row[:], ges[k])
            gb_sb = xw.tile([P, C], F32, tag="gb")
            for co, cw in ctiles:
                pgb = pm_pool.tile([P, 512], F32, tag="pt")
                nc.tensor.matmul(pgb[:, :cw], ones_row[:], g_row[0:1, co:co + cw],
                                 start=True, stop=True)
                nc.vector.tensor_copy(gb_sb[:, co:co + cw], pgb[:, :cw])

            # group col tiles in pairs so each weight load serves 2 matmuls
            ctpairs = [ctiles[i:i + 2] for i in range(0, len(ctiles), 2)]
            z_sb = zw.tile([P, LT, C], F32R, tag="z")
            for lt in range(LT):
                lsl = slice(lt * P, (lt + 1) * P)
                for pi, pair in enumerate(ctpairs):
                    phts = [ph_pool.tile([P, 512], F32, tag="ph",
                                         name=f"ph{k}_{lt}_{pi}_{j}")
                            for j in range(len(pair))]
                    for hc in range(HC):
                        for j, (co, cw) in enumerate(pair):
                            nc.tensor.matmul(
                                phts[j][:, :cw], w1_sb[:, hc, lsl],
                                xe_r[hc][:, co:co + cw],
                                start=(hc == 0), stop=(hc == HC - 1))
                    for j, (co, cw) in enumerate(pair):
                        csl = slice(co, co + cw)
                        nc.scalar.activation(z_sb[:, lt, csl], phts[j][:, :cw],
                                             AF.Silu,
                                             bias=rb1_sb[:, k, lt:lt + 1])
                        nc.vector.tensor_mul(z_sb[:, lt, csl],
                                             z_sb[:, lt, csl], gb_sb[:, csl])

            for ht in range(HT):
                hsl = slice(ht * P, (ht + 1) * P)
                for pi, pair in enumerate(ctpairs):
                    pots = [po_pool.tile([P, 512], F32, tag="po",
                                         name=f"po{k}_{ht}_{pi}_{j}")
                            for j in range(len(pair))]
                    for lc in range(LT):
                        for j, (co, cw) in enumerate(pair):
                            nc.tensor.matmul(
                                pots[j][:, :cw], w2_sb[:, lc, hsl],
                                z_sb[:, lc, co:co + cw],
                                start=(lc == 0), stop=(lc == LT - 1))
                    for j, (co, cw) in enumerate(pair):
                        csl = slice(co, co + cw)
                        stg = small.tile([P, 512], F32, tag="stg")
                        nc.vector.tensor_copy(stg[:, :cw], pots[j][:, :cw])
                        eng = nc.sync if (ht + pi + j) % 2 == 0 else nc.scalar
                        eng.dma_start(yes[k][ht * P:(ht + 1) * P, csl],
                                      stg[:, :cw])

    nc.compile()
    return nc


_NC_CACHE = {}


def _get(name, builder, *args):
    key = (name,) + args
    if key not in _NC_CACHE:
        _NC_CACHE[key] = builder(*args)
    return _NC_CACHE[key]


def _prep_host(inputs):
    (hidden_states, expert_centroids, expert_biases, sw1, sb1, sw2, sb2,
     rw1, rb1, rw2, rb2) = inputs
    flat = hidden_states.reshape(T_FULL, H)
    prep = {
        "flat": flat,
        "ct": np.ascontiguousarray(expert_centroids.T),
        "eb": np.ascontiguousarray(expert_biases.reshape(E, 1)),
        "s1": np.ascontiguousarray(sw1.sum(axis=0)),
        "s2": np.ascontiguousarray(sw2.sum(axis=0)),
        "sb1": np.ascontiguousarray(sb1.sum(axis=0).reshape(LT, P).T),
        "sb2": np.ascontiguousarray(sb2.sum(axis=0).reshape(HT, P).T),
        "rb1": np.ascontiguousarray(rb1.reshape(E, LT, P).transpose(0, 2, 1)),
        "rb2": rb2, "rw1": rw1, "rw2": rw2,
        "rb2x": np.ascontiguousarray(
            np.concatenate([rb2, sb2.sum(axis=0)[None]], axis=0)),
    }
    return prep


def _aux_from_stats(mask_sum, sp_sum):
    f_i = mask_sum * (E / (TOP_K * S)) / B
    p_i = sp_sum / T_FULL
    return np.float32(BALANCE_ALPHA * float((f_i * p_i).sum()))


def _perm_h(w):
    """[H, N] -> partition-major [P, HC, N] (contiguous per partition)."""
    return np.ascontiguousarray(w.reshape(-1, P, w.shape[-1]).transpose(1, 0, 2))


def _kernel_ep2(prep):
    zero_bias = (not prep["eb"].any()) and (not prep["rb2x"].any())
    nc1 = _get("s1", _build_stage1, zero_bias)
    flatT_perm = _perm_h(np.ascontiguousarray(prep["flat"].T))  # [P, HC, TF]
    ct_p = _perm_h(prep["ct"])
    s1_p = _perm_h(prep["s1"])
    s2_p = _perm_h(prep["s2"])
    in1 = []
    for c in range(N_CORES):
        xt_c = flatT_perm[:, :, c * T:(c + 1) * T]
        m = {"xta": np.ascontiguousarray(xt_c[:, 0:3]),
             "xtb": np.ascontiguousarray(xt_c[:, 3:6]),
             "xtc": np.ascontiguousarray(xt_c[:, 6:8]),
             "ct": ct_p, "s1": s1_p, "s2": s2_p, "sb1": prep["sb1"]}
        if not zero_bias:
            m["eb"] = prep["eb"]
            m["rb2"] = prep["rb2x"]
        in1.append(m)
    res1 = run_bass_kernel_spmd(nc1, in1, core_ids=list(range(N_CORES)))

    mask_sum = np.zeros(E, dtype=np.float64)
    sp_sum = np.zeros(E, dtype=np.float64)
    gT_full = np.empty((E, T_FULL), dtype=np.float32)
    out = np.empty((T_FULL, H), dtype=np.float32)
    for c in range(N_CORES):
        r = res1.results[c]
        out[c * T:(c + 1) * T] = r["yt"].T
        gT_full[:, c * T:(c + 1) * T] = r["gt_out"]
        mask_sum += r["stats"][:, 0]
        sp_sum += r["stats"][:, 1]
    aux = _aux_from_stats(mask_sum, sp_sum)

    # token dispatch on the device-computed top-k gates; rank experts by
    # load so the low-count slot can use a smaller capacity
    idx = [np.nonzero(gT_full[e] > 0)[0] for e in range(E)]
    order = sorted(range(E), key=lambda e: -len(idx[e]))
    # core c: slot 0 <- rank c (heavy), slot 1 <- rank 15-c (light)
    assign = [(order[c], order[E - 1 - c]) for c in range(N_CORES)]
    rup = lambda n: max(512, -(-n // 256) * 256)
    C1 = rup(max(len(idx[e]) for e, _ in assign))
    C2 = rup(max(len(idx[e]) for _, e in assign))

    w1_perm = np.ascontiguousarray(
        prep["rw1"].reshape(E, HC, P, L).transpose(0, 2, 1, 3))
    w2_perm = np.ascontiguousarray(
        prep["rw2"].reshape(E, LT, P, H).transpose(0, 2, 1, 3))

    nc2 = _get("s2", _build_stage2, C1, C2)
    caps = (C1, C2)
    in2 = []
    for c in range(N_CORES):
        m = {}
        for k in range(EPC):
            e = assign[c][k]
            n = len(idx[e])
            xe = np.zeros((P, HC, caps[k]), dtype=np.float32)
            gevals = np.zeros((1, caps[k]), dtype=np.float32)
            xe[:, :, :n] = flatT_perm[:, :, idx[e]]
            gevals[0, :n] = gT_full[e, idx[e]]
            m[f"xe{k}"] = xe
            m[f"ge{k}"] = gevals
        es = list(assign[c])
        m["w1p"] = np.ascontiguousarray(w1_perm[es])
        m["w2p"] = np.ascontiguousarray(w2_perm[es])
        m["rb1p"] = np.ascontiguousarray(prep["rb1"][es])
        in2.append(m)
    res2 = run_bass_kernel_spmd(nc2, in2, core_ids=list(range(N_CORES)))

    for c in range(N_CORES):
        for k in range(EPC):
            e = assign[c][k]
            n = len(idx[e])
            out[idx[e]] += res2.results[c][f"ye{k}"][:, :n].T
    return out.reshape(B, S, H), aux


def _kernel_dense(prep):
    nc = _get("dense", _build_dense)
    in_maps = []
    for c in range(N_CORES):
        xt_c = np.ascontiguousarray(prep["flat"][c * T:(c + 1) * T].T)
        in_maps.append({
            "xt": xt_c, "ct": prep["ct"], "eb": prep["eb"],
            "w1": prep["rw1"], "w2": prep["rw2"], "s1": prep["s1"],
            "s2": prep["s2"], "rb1": prep["rb1"], "rb2": prep["rb2"],
            "sb1": prep["sb1"], "sb2": prep["sb2"],
        })
    res = run_bass_kernel_spmd(nc, in_maps, core_ids=list(range(N_CORES)))

    out = np.empty((T_FULL, H), dtype=np.float32)
    mask_sum = np.zeros(E, dtype=np.float64)
    sp_sum = np.zeros(E, dtype=np.float64)
    for c in range(N_CORES):
        out[c * T:(c + 1) * T] = res.results[c]["yt"].T
        st = res.results[c]["stats"]
        mask_sum += st[:, 0]
        sp_sum += st[:, 1]
    return out.reshape(B, S, H), _aux_from_stats(mask_sum, sp_sum)


def _build_dense():
    """Single-launch fallback: every core runs all experts on its shard."""
    nc = bacc.Bacc("TRN2", target_bir_lowering=False, debug=False,
                   num_devices=N_CORES)

    xt = nc.dram_tensor("xt", [H, T], F32, kind="ExternalInput").ap()
    ct = nc.dram_tensor("ct", [H, E], F32, kind="ExternalInput").ap()
    eb = nc.dram_tensor("eb", [E, 1], F32, kind="ExternalInput").ap()
    w1 = nc.dram_tensor("w1", [E, H, L], F32, kind="ExternalInput").ap()
    w2 = nc.dram_tensor("w2", [E, L, H], F32, kind="ExternalInput").ap()
    s1 = nc.dram_tensor("s1", [H, L], F32, kind="ExternalInput").ap()
    s2 = nc.dram_tensor("s2", [L, H], F32, kind="ExternalInput").ap()
    rb1 = nc.dram_tensor("rb1", [E, P, LT], F32, kind="ExternalInput").ap()
    rb2 = nc.dram_tensor("rb2", [E, H], F32, kind="ExternalInput").ap()
    sb1 = nc.dram_tensor("sb1", [P, LT], F32, kind="ExternalInput").ap()
    sb2 = nc.dram_tensor("sb2", [P, HT], F32, kind="ExternalInput").ap()
    yt = nc.dram_tensor("yt", [H, T], F32, kind="ExternalOutput").ap()
    stats = nc.dram_tensor("stats", [E, 2], F32, kind="ExternalOutput").ap()

    with tile.TileContext(nc) as tc, ExitStack() as ctx:
        const = ctx.enter_context(tc.tile_pool(name="const", bufs=1))
        xpool = ctx.enter_context(tc.tile_pool(name="xpool", bufs=1))
        work = ctx.enter_context(tc.tile_pool(name="work", bufs=2))
        small = ctx.enter_context(tc.tile_pool(name="small", bufs=4))
        ph_pool = ctx.enter_context(tc.tile_pool(name="ph", bufs=2, space="PSUM"))
        po_pool = ctx.enter_context(tc.tile_pool(name="po", bufs=2, space="PSUM"))
        pm_pool = ctx.enter_context(tc.tile_pool(name="pm", bufs=2, space="PSUM"))
        ps_pool = ctx.enter_context(tc.tile_pool(name="ps", bufs=1, space="PSUM"))

        identity = const.tile([P, P], F32)
        make_identity(nc, identity)
        ones_col = const.tile([P, 1], F32)
        nc.any.memset(ones_col[:], 1.0)
        ones_row = const.tile([1, P], F32)
        nc.any.memset(ones_row[:], 1.0)

        xt_sb = xpool.tile([P, HC, T], F32, tag="x")
        nc.sync.dma_start(xt_sb[:], xt.rearrange("(c p) t -> p c t", p=P))
        ct_sb = const.tile([P, HC, E], F32)
        nc.sync.dma_start(ct_sb[:], ct.rearrange("(c p) e -> p c e", p=P))
        eb_sb = const.tile([E, 1], F32)
        nc.sync.dma_start(eb_sb[:], eb[:])
        rb2_sb = const.tile([E, H], F32)
        nc.sync.dma_start(rb2_sb[:], rb2[:])
        rb1_sb = const.tile([P, E, LT], F32)
        nc.sync.dma_start(rb1_sb[:], rb1.rearrange("e p l -> p e l"))
        sb1_sb = const.tile([P, LT], F32)
        nc.sync.dma_start(sb1_sb[:], sb1[:])
        sb2_sb = const.tile([P, HT], F32)
        nc.sync.dma_start(sb2_sb[:], sb2[:])

        acc = const.tile([P, HT, T], F32)
        gT = const.tile([E, T], F32)

        _router_and_gates(nc, tc, ctx, (small, pm_pool, ps_pool), xt_sb,
                          ct_sb, eb_sb, ones_col, identity, gT, stats)

        for ht in range(HT):
            hsl = slice(ht * P, (ht + 1) * P)
            for half in range(NH):
                csl = slice(half * 512, (half + 1) * 512)
                pc = po_pool.tile([P, 512], F32, tag="po")
                nc.tensor.matmul(pc[:], rb2_sb[:, hsl], gT[:, csl],
                                 start=True, stop=True)
                nc.vector.tensor_add(acc[:, ht, csl], xt_sb[:, ht, csl], pc[:])
                nc.vector.tensor_scalar_add(acc[:, ht, csl], acc[:, ht, csl],
                                            sb2_sb[:, ht:ht + 1])

        xt_r = xpool.tile([P, HC, T], F32R, tag="x")
        nc.gpsimd.dma_start(xt_r[:], xt.rearrange("(c p) t -> p c t", p=P))

        for e in range(E + 1):
            shared = (e == E)
            w1_sb = work.tile([P, HC, L], F32R, tag="w1")
            w2_sb = work.tile([P, LT, H], F32R, tag="w2")
            if shared:
                nc.gpsimd.dma_start(w1_sb[:], s1.rearrange("(c p) l -> p c l", p=P))
                nc.gpsimd.dma_start(w2_sb[:], s2.rearrange("(c p) h -> p c h", p=P))
            else:
                nc.gpsimd.dma_start(w1_sb[:], w1[e].rearrange("(c p) l -> p c l", p=P))
                nc.gpsimd.dma_start(w2_sb[:], w2[e].rearrange("(c p) h -> p c h", p=P))

            if not shared:
                gb_sb = work.tile([P, T], F32, tag="gb")
                g_row = work.tile([1, T], F32, tag="g_row")
                nc.sync.dma_start(g_row[:], gT[e:e + 1, :])
                for half in range(NH):
                    csl = slice(half * 512, (half + 1) * 512)
                    pgb = pm_pool.tile([P, 512], F32, tag="pt")
                    nc.tensor.matmul(pgb[:], ones_row[:], g_row[0:1, csl],
                                     start=True, stop=True)
                    nc.vector.tensor_copy(gb_sb[:, csl], pgb[:])

            z_sb = work.tile([P, LT, T], F32R, tag="z")
            for lt in range(LT):
                lsl = slice(lt * P, (lt + 1) * P)
                for half in range(NH):
                    csl = slice(half * 512, (half + 1) * 512)
                    pht = ph_pool.tile([P, 512], F32, tag="ph")
                    for hc in range(HC):
                        nc.tensor.matmul(pht[:], w1_sb[:, hc, lsl],
                                         xt_r[:, hc, csl],
                                         start=(hc == 0), stop=(hc == HC - 1))
                    bias_ap = (sb1_sb if shared else rb1_sb[:, e, :])[:, lt:lt + 1]
                    nc.scalar.activation(z_sb[:, lt, csl], pht[:], AF.Silu,
                                         bias=bias_ap)
                    if not shared:
                        nc.vector.tensor_mul(z_sb[:, lt, csl], z_sb[:, lt, csl],
                                             gb_sb[:, csl])

            for ht in range(HT):
                hsl = slice(ht * P, (ht + 1) * P)
                for half in range(NH):
                    csl = slice(half * 512, (half + 1) * 512)
                    pot = po_pool.tile([P, 512], F32, tag="po")
                    for lc in range(LT):
                        nc.tensor.matmul(pot[:], w2_sb[:, lc, hsl],
                                         z_sb[:, lc, csl],
                                         start=(lc == 0), stop=(lc == LT - 1))
                    nc.vector.tensor_add(acc[:, ht, csl], acc[:, ht, csl], pot[:])

        for ht in range(HT):
            nc.sync.dma_start(yt[ht * P:(ht + 1) * P, :], acc[:, ht, :])

    nc.compile()
    return nc


def kernel(hidden_states, expert_centroids, expert_biases, sw1, sb1, sw2, sb2,
           rw1, rb1, rw2, rb2, **_ignored):
    args = [np.asarray(a, dtype=np.float32) for a in
            (hidden_states, expert_centroids, expert_biases, sw1, sb1, sw2,
             sb2, rw1, rb1, rw2, rb2)]
    prep = _prep_host(args)
    strategy = os.environ.get("MOE_STRATEGY", "ep2")
    if strategy == "dense":
        return _kernel_dense(prep)
    return _kernel_ep2(prep)


# revision 49
# speedup vs baseline: 1.0474x; 1.0152x over previous
"""DeepSeek-MoE (16 routed experts, top-2, 1 shared expert) on 8 Trainium2 cores.

Default strategy "ep2" (expert-parallel, two launches):
  Stage 1 (data-parallel over tokens): each core takes 1024 of the 8192
  tokens and computes the router (exact fp32), top-2 gates, balance-loss
  stats, plus the shared expert and residual:  y1 = x + shared(x) +
  sum_e g_e*rb2[e] + sb2.  It returns the full gate matrix.
  Host dispatch: tokens are packed per expert from the device-computed
  top-k gates (the "all-to-all" of the sharding hint, done host-side since
  the kernel contract is full-input/full-output).
  Stage 2 (expert-parallel): core c holds routed experts 2c and 2c+1 and
  runs them densely over just their assigned (padded) tokens, applying the
  gate to the silu'd intermediate before the second matmul. Host adds the
  gathered expert outputs into y1.

Big matmuls run as float32r (full-rate fp32 PE mode, ~1e-4 rel err); the
router runs exact fp32 so top-2 selection matches the reference.

Fallback strategy "dense" (MOE_STRATEGY=dense): single launch, each core
computes all 16 experts densely for its token shard.

Shapes hardcoded per the problem spec: hidden_states [4, 2048, 1024],
E=16, L=512, H=1024, top-2.
"""

import os
import sys

for _p in ("/opt/trn_rl_repo", "/root/.axon_site/_ro/trn_rl_repo"):
    if _p not in sys.path:
        sys.path.append(_p)

import numpy as np
from contextlib import ExitStack

import concourse.bass as bass
import concourse.tile as tile
from concourse import bacc, mybir
from concourse.bass_utils import run_bass_kernel_spmd
from concourse.masks import make_identity

F32 = mybir.dt.float32
F32R = mybir.dt.float32r
AF = mybir.ActivationFunctionType
ALU = mybir.AluOpType

P = 128
B, S, H, L, E = 4, 2048, 1024, 512, 16
TOP_K = 2
BALANCE_ALPHA = 0.001
N_CORES = 8
T_FULL = B * S
T = T_FULL // N_CORES  # tokens per core in stage 1 (1024)
HC = H // P  # 8 contraction chunks over H
LT = L // P  # 4 tiles over L
HT = H // P  # 8 tiles over H
NH = T // 512  # moving-dim halves (2)
EPC = E // N_CORES  # routed experts per core in stage 2 (2)


def _col_tiles(c):
    """Split token-column count c into moving-dim tiles of <=512."""
    out, o = [], 0
    while o < c:
        w = min(512, c - o)
        out.append((o, w))
        o += w
    return out


def _router_and_gates(nc, tc, ctx, pools, xt_sb, ct_sb, eb_sb, ones_col,
                      identity, gT, stats_ap):
    """Emit router matmuls (exact fp32), top-2 gating, and balance stats.

    Fills gT [E, T] in SBUF and DMAs stats [E, 2] (mask row-sums, s' row-
    sums) to DRAM.
    """
    small, pm_pool, ps_pool = pools
    aff_T = ctx.enter_context(tc.tile_pool(name="aff", bufs=1))
    aff = aff_T.tile([E, T], F32)
    biased = aff_T.tile([E, T], F32) if eb_sb is not None else aff

    for half in range(NH):
        pa = pm_pool.tile([E, 512], F32, tag="pt")
        for hc in range(HC):
            nc.tensor.matmul(pa[:], ct_sb[:, hc, :],
                             xt_sb[:, hc, half * 512:(half + 1) * 512],
                             start=(hc == 0), stop=(hc == HC - 1))
        nc.scalar.activation(aff[:, half * 512:(half + 1) * 512], pa[:],
                             AF.Sigmoid)
    if eb_sb is not None:
        nc.vector.tensor_scalar_add(biased[:], aff[:], eb_sb[:, 0:1])

    pmask_sum = ps_pool.tile([E, 1], F32)
    psp_sum = ps_pool.tile([E, 1], F32)
    for tt in range(T // P):
        tsl = slice(tt * P, (tt + 1) * P)
        pta = pm_pool.tile([P, E], F32, tag="pt")
        nc.tensor.transpose(pta[:], aff[:, tsl], identity[0:E, 0:E])
        afftok = small.tile([P, E], F32)
        nc.vector.tensor_copy(afftok[:], pta[:])
        if eb_sb is not None:
            ptb = pm_pool.tile([P, E], F32, tag="pt")
            nc.tensor.transpose(ptb[:], biased[:, tsl], identity[0:E, 0:E])
            biastok = small.tile([P, E], F32)
            nc.vector.tensor_copy(biastok[:], ptb[:])
        else:
            biastok = afftok

        mx8 = small.tile([P, 8], F32)
        nc.vector.max(mx8[:], biastok[:])
        mask = small.tile([P, E], F32)
        nc.vector.tensor_scalar(mask[:], biastok[:], mx8[:, 1:2], None,
                                op0=ALU.is_ge)
        sel = small.tile([P, E], F32)
        nc.vector.tensor_mul(sel[:], afftok[:], mask[:])
        den = small.tile([P, 1], F32)
        nc.vector.reduce_sum(den[:], sel[:], axis=mybir.AxisListType.X)
        nc.vector.tensor_scalar_add(den[:], den[:], 1e-8)
        rec = small.tile([P, 1], F32)
        nc.vector.reciprocal(rec[:], den[:])
        g_tok = small.tile([P, E], F32)
        nc.vector.tensor_scalar_mul(g_tok[:], sel[:], rec[:, 0:1])

        den2 = small.tile([P, 1], F32)
        nc.vector.reduce_sum(den2[:], afftok[:], axis=mybir.AxisListType.X)
        nc.vector.tensor_scalar_add(den2[:], den2[:], 1e-8)
        rec2 = small.tile([P, 1], F32)
        nc.vector.reciprocal(rec2[:], den2[:])
        sprime = small.tile([P, E], F32)
        nc.vector.tensor_scalar_mul(sprime[:], afftok[:], rec2[:, 0:1])

        nc.tensor.matmul(pmask_sum[:], mask[:], ones_col[:],
                         start=(tt == 0), stop=(tt == T // P - 1))
        nc.tensor.matmul(psp_sum[:], sprime[:], ones_col[:],
                         start=(tt == 0), stop=(tt == T // P - 1))

        ptg = pm_pool.tile([E, P], F32, tag="pt")
        nc.tensor.transpose(ptg[:], g_tok[:], identity[:])
        nc.vector.tensor_copy(gT[:, tsl], ptg[:])

    stats_sb = small.tile([E, 2], F32)
    nc.vector.tensor_copy(stats_sb[:, 0:1], pmask_sum[:])
    nc.vector.tensor_copy(stats_sb[:, 1:2], psp_sum[:])
    nc.sync.dma_start(stats_ap[:], stats_sb[:])


def _build_stage1(zero_bias=True):
    """Router + gates + stats + shared expert + residual, DP over tokens.

    zero_bias=True omits the expert_biases add and the rb2/sb2 correction
    (all zero in this model); the False variant keeps the exact math.
    """
    nc = bacc.Bacc("TRN2", target_bir_lowering=False, debug=False,
                   num_devices=N_CORES)

    xta = nc.dram_tensor("xta", [P, 3, T], F32, kind="ExternalInput").ap()
    xtb = nc.dram_tensor("xtb", [P, 3, T], F32, kind="ExternalInput").ap()
    xtc = nc.dram_tensor("xtc", [P, 2, T], F32, kind="ExternalInput").ap()
    ct = nc.dram_tensor("ct", [P, HC, E], F32, kind="ExternalInput").ap()
    s1 = nc.dram_tensor("s1", [P, HC, L], F32, kind="ExternalInput").ap()
    s2 = nc.dram_tensor("s2", [P, LT, H], F32, kind="ExternalInput").ap()
    sb1 = nc.dram_tensor("sb1", [P, LT], F32, kind="ExternalInput").ap()
    if not zero_bias:
        eb = nc.dram_tensor("eb", [E, 1], F32, kind="ExternalInput").ap()
        rb2 = nc.dram_tensor("rb2", [E + 1, H], F32, kind="ExternalInput").ap()
    yt = nc.dram_tensor("yt", [H, T], F32, kind="ExternalOutput").ap()
    gt_out = nc.dram_tensor("gt_out", [E, T], F32, kind="ExternalOutput").ap()
    stats = nc.dram_tensor("stats", [E, 2], F32, kind="ExternalOutput").ap()

    with tile.TileContext(nc) as tc, ExitStack() as ctx:
        const = ctx.enter_context(tc.tile_pool(name="const", bufs=1))
        xpool = ctx.enter_context(tc.tile_pool(name="xpool", bufs=1))
        work = ctx.enter_context(tc.tile_pool(name="work", bufs=1))
        small = ctx.enter_context(tc.tile_pool(name="small", bufs=4))
        ph_pool = ctx.enter_context(tc.tile_pool(name="ph", bufs=2, space="PSUM"))
        po_pool = ctx.enter_context(tc.tile_pool(name="po", bufs=2, space="PSUM"))
        pm_pool = ctx.enter_context(tc.tile_pool(name="pm", bufs=2, space="PSUM"))
        ps_pool = ctx.enter_context(tc.tile_pool(name="ps", bufs=1, space="PSUM"))

        identity = const.tile([P, P], F32)
        make_identity(nc, identity)
        ones_col = const.tile([P, 1], F32)
        nc.any.memset(ones_col[:], 1.0)

        # PE warm-up: a few throwaway matmuls during the DMA lead push the
        # HAM clock gate to full rate before the fp32 router matmuls issue
        warm = const.tile([P, 512], F32)
        nc.any.memset(warm[:], 0.0)
        pwarm = pm_pool.tile([P, 512], F32, tag="pt")
        for _ in range(4):
            nc.tensor.matmul(pwarm[:], warm[:, 0:P], warm[:],
                             start=True, stop=True)

        # x (exact fp32) split across both HW DGE queues (SP + ACT);
        # each piece is partition-contiguous in DRAM
        xt_sb = xpool.tile([P, HC, T], F32, tag="x")
        nc.sync.dma_start(xt_sb[:, 0:3, :], xta[:])
        nc.scalar.dma_start(xt_sb[:, 3:6, :], xtb[:])
        nc.sync.dma_start(xt_sb[:, 6:8, :], xtc[:])
        ct_sb = const.tile([P, HC, E], F32)
        nc.scalar.dma_start(ct_sb[:], ct[:])
        sb1_sb = const.tile([P, LT], F32)
        nc.sync.dma_start(sb1_sb[:], sb1[:])

        # shared-expert operands stream on the gpsimd (casting) queue in
        # parallel with the fp32 x load: w1 then per-chunk f32r x so the
        # shared expert's first matmuls start before the router's x lands
        w1_sb = work.tile([P, HC, L], F32R, tag="w1")
        for q in range(4):
            nc.gpsimd.dma_start(w1_sb[:, 2 * q:2 * q + 2, :],
                                s1[:, 2 * q:2 * q + 2, :])
        xt_r = [const.tile([P, T], F32R, name=f"xtr{hc}") for hc in range(HC)]
        for hc in range(HC):
            if hc < 3:
                nc.gpsimd.dma_start(xt_r[hc][:], xta[:, hc, :])
            elif hc < 6:
                nc.gpsimd.dma_start(xt_r[hc][:], xtb[:, hc - 3, :])
            else:
                nc.gpsimd.dma_start(xt_r[hc][:], xtc[:, hc - 6, :])
        w2_sb = work.tile([P, LT, H], F32R, tag="w2")
        for q in range(4):
            nc.gpsimd.dma_start(w2_sb[:, q, :], s2[:, q, :])

        acc = const.tile([P, HT, T], F32)
        halves = [slice(h * 512, (h + 1) * 512) for h in range(NH)]

        # shared-expert first matmul chain emitted BEFORE the router: its
        # f32r operands stream on gpsimd while the exact-fp32 x is still
        # loading, so the PE has work during the router's DMA wait
        z_sb = work.tile([P, LT, T], F32R, tag="z")
        for lt in range(LT):
            lsl = slice(lt * P, (lt + 1) * P)
            phts = [ph_pool.tile([P, 512], F32, tag="ph", name=f"ph{lt}_{h}")
                    for h in range(NH)]
            for hc in range(HC):
                for h, csl in enumerate(halves):
                    nc.tensor.matmul(phts[h][:], w1_sb[:, hc, lsl],
                                     xt_r[hc][:, csl],
                                     start=(hc == 0), stop=(hc == HC - 1))
            for h, csl in enumerate(halves):
                nc.scalar.activation(z_sb[:, lt, csl], phts[h][:], AF.Silu,
                                     bias=sb1_sb[:, lt:lt + 1])

        if zero_bias:
            gT = const.tile([E, T], F32)
            eb_sb = None
        else:
            eb_sb = const.tile([E, 1], F32)
            nc.sync.dma_start(eb_sb[:], eb[:])
            rb2_sb = const.tile([E + 1, H], F32)
            nc.sync.dma_start(rb2_sb[:], rb2[:])
            gTx = const.tile([E + 1, T], F32)
            gT = gTx[0:E, :]
            # row E stays 1.0 (gate for the sb2 bias row)
            nc.any.memset(gTx[:], 1.0)

        _router_and_gates(nc, tc, ctx, (small, pm_pool, ps_pool), xt_sb,
                          ct_sb, eb_sb, ones_col, identity, gT, stats)
        nc.sync.dma_start(gt_out[:], gT[:])

        if not zero_bias:
            # acc = x + sum_e g_e*rb2[e] + sb2   (sb2 rides as gate-1 row E)
            for ht in range(HT):
                hsl = slice(ht * P, (ht + 1) * P)
                for half in range(NH):
                    csl = slice(half * 512, (half + 1) * 512)
                    pc = po_pool.tile([P, 512], F32, tag="po")
                    nc.tensor.matmul(pc[:], rb2_sb[:, hsl], gTx[:, csl],
                                     start=True, stop=True)
                    nc.vector.tensor_add(acc[:, ht, csl], xt_sb[:, ht, csl],
                                         pc[:])

        for ht in range(HT):
            hsl = slice(ht * P, (ht + 1) * P)
            pots = [po_pool.tile([P, 512], F32, tag="po", name=f"po{ht}_{h}")
                    for h in range(NH)]
            for lc in range(LT):
                for h, csl in enumerate(halves):
                    nc.tensor.matmul(pots[h][:], w2_sb[:, lc, hsl],
                                     z_sb[:, lc, csl],
                                     start=(lc == 0), stop=(lc == LT - 1))
            src_ap = xt_sb if zero_bias else acc
            for h, csl in enumerate(halves):
                nc.vector.tensor_add(acc[:, ht, csl], src_ap[:, ht, csl],
                                     pots[h][:])

        # stores split across both HW DGE queues
        for ht in range(HT):
            eng = nc.sync if ht % 2 == 0 else nc.scalar
            eng.dma_start(yt[ht * P:(ht + 1) * P, :], acc[:, ht, :])

    nc.compile()
    return nc


def _build_stage2(C1, C2):
    """Two routed experts per core over their gathered (padded) tokens.

    Slot 0 holds a high-count expert (capacity C1), slot 1 a low-count
    one (C2 <= C1) — the host ranks experts by load so padding is small.
    """
    nc = bacc.Bacc("TRN2", target_bir_lowering=False, debug=False,
                   num_devices=N_CORES)

    caps = (C1, C2)
    xes = [nc.dram_tensor(f"xe{k}", [P, HC, caps[k]], F32,
                          kind="ExternalInput").ap() for k in range(EPC)]
    ges = [nc.dram_tensor(f"ge{k}", [1, caps[k]], F32,
                          kind="ExternalInput").ap() for k in range(EPC)]
    w1p = nc.dram_tensor("w1p", [EPC, P, HC, L], F32, kind="ExternalInput").ap()
    w2p = nc.dram_tensor("w2p", [EPC, P, LT, H], F32, kind="ExternalInput").ap()
    rb1p = nc.dram_tensor("rb1p", [EPC, P, LT], F32, kind="ExternalInput").ap()
    yes = [nc.dram_tensor(f"ye{k}", [H, caps[k]], F32,
                          kind="ExternalOutput").ap() for k in range(EPC)]

    with tile.TileContext(nc) as tc, ExitStack() as ctx:
        const = ctx.enter_context(tc.tile_pool(name="const", bufs=1))
        xep = ctx.enter_context(tc.tile_pool(name="xep", bufs=16))
        xw = ctx.enter_context(tc.tile_pool(name="xw", bufs=2))
        zw = ctx.enter_context(tc.tile_pool(name="zw", bufs=1))
        small = ctx.enter_context(tc.tile_pool(name="small", bufs=4))
        ph_pool = ctx.enter_context(tc.tile_pool(name="ph", bufs=3, space="PSUM"))
        po_pool = ctx.enter_context(tc.tile_pool(name="po", bufs=3, space="PSUM"))
        pm_pool = ctx.enter_context(tc.tile_pool(name="pm", bufs=2, space="PSUM"))

        ones_row = const.tile([1, P], F32)
        nc.any.memset(ones_row[:], 1.0)
        rb1_sb = const.tile([P, EPC, LT], F32)
        nc.sync.dma_start(rb1_sb[:], rb1p.rearrange("k p l -> p k l"))

        warm = const.tile([P, 512], F32)
        nc.any.memset(warm[:], 0.0)
        pwarm = pm_pool.tile([P, 512], F32, tag="pt")
        for _ in range(4):
            nc.tensor.matmul(pwarm[:], warm[:, 0:P], warm[:],
                             start=True, stop=True)

        for k in range(EPC):
            C = caps[k]
            ctiles = _col_tiles(C)
            w1_sb = xw.tile([P, HC, L], F32R, tag="w1")
            for q in range(4):
                nc.gpsimd.dma_start(w1_sb[:, 2 * q:2 * q + 2, :],
                                    w1p[k][:, 2 * q:2 * q + 2, :])
            # per-chunk token tiles: lets expert k+1's loads start while
            # expert k's first matmul chain is still reading its chunks
            xe_r = [xep.tile([P, C], F32R, tag="xec", name=f"xec{k}_{hc}")
                    for hc in range(HC)]
            for hc in range(HC):
                nc.gpsimd.dma_start(xe_r[hc][:], xes[k][:, hc, :])
            w2_sb = xw.tile([P, LT, H], F32R, tag="w2")
            for q in range(4):
                nc.gpsimd.dma_start(w2_sb[:, q, :], w2p[k][:, q, :])

            g_row = xw.tile([1, C], F32, tag="g_row")
            nc.sync.dma_start(g_row[:], ges[k])
            gb_sb = xw.tile([P, C], F32, tag="gb")
            for co, cw in ctiles:
                pgb = pm_pool.tile([P, 512], F32, tag="pt")
                nc.tensor.matmul(pgb[:, :cw], ones_row[:], g_row[0:1, co:co + cw],
                                 start=True, stop=True)
                nc.vector.tensor_copy(gb_sb[:, co:co + cw], pgb[:, :cw])

            # group col tiles in pairs so each weight load serves 2 matmuls
            ctpairs = [ctiles[i:i + 2] for i in range(0, len(ctiles), 2)]
            z_sb = zw.tile([P, LT, C], F32R, tag="z")
            for lt in range(LT):
                lsl = slice(lt * P, (lt + 1) * P)
                for pi, pair in enumerate(ctpairs):
                    phts = [ph_pool.tile([P, 512], F32, tag="ph",
                                         name=f"ph{k}_{lt}_{pi}_{j}")
                            for j in range(len(pair))]
                    for hc in range(HC):
                        for j, (co, cw) in enumerate(pair):
                            nc.tensor.matmul(
                                phts[j][:, :cw], w1_sb[:, hc, lsl],
                                xe_r[hc][:, co:co + cw],
                                start=(hc == 0), stop=(hc == HC - 1))
                    for j, (co, cw) in enumerate(pair):
                        csl = slice(co, co + cw)
                        nc.scalar.activation(z_sb[:, lt, csl], phts[j][:, :cw],
                                             AF.Silu,
                                             bias=rb1_sb[:, k, lt:lt + 1])
                        nc.vector.tensor_mul(z_sb[:, lt, csl],
                                             z_sb[:, lt, csl], gb_sb[:, csl])

            for ht in range(HT):
                hsl = slice(ht * P, (ht + 1) * P)
                for pi, pair in enumerate(ctpairs):
                    pots = [po_pool.tile([P, 512], F32, tag="po",
                                         name=f"po{k}_{ht}_{pi}_{j}")
                            for j in range(len(pair))]
                    for lc in range(LT):
                        for j, (co, cw) in enumerate(pair):
                            nc.tensor.matmul(
                                pots[j][:, :cw], w2_sb[:, lc, hsl],
                                z_sb[:, lc, co:co + cw],
                                start=(lc == 0), stop=(lc == LT - 1))
                    for j, (co, cw) in enumerate(pair):
                        csl = slice(co, co + cw)
                        stg = small.tile([P, 512], F32, tag="stg")
                        nc.vector.tensor_copy(stg[:, :cw], pots[j][:, :cw])
                        eng = nc.sync if (ht + pi + j) % 2 == 0 else nc.scalar
                        eng.dma_start(yes[k][ht * P:(ht + 1) * P, csl],
                                      stg[:, :cw])

    nc.compile()
    return nc


_NC_CACHE = {}


def _get(name, builder, *args):
    key = (name,) + args
    if key not in _NC_CACHE:
        _NC_CACHE[key] = builder(*args)
    return _NC_CACHE[key]


def _prep_host(inputs):
    (hidden_states, expert_centroids, expert_biases, sw1, sb1, sw2, sb2,
     rw1, rb1, rw2, rb2) = inputs
    flat = hidden_states.reshape(T_FULL, H)
    prep = {
        "flat": flat,
        "ct": np.ascontiguousarray(expert_centroids.T),
        "eb": np.ascontiguousarray(expert_biases.reshape(E, 1)),
        "s1": np.ascontiguousarray(sw1.sum(axis=0)),
        "s2": np.ascontiguousarray(sw2.sum(axis=0)),
        "sb1": np.ascontiguousarray(sb1.sum(axis=0).reshape(LT, P).T),
        "sb2": np.ascontiguousarray(sb2.sum(axis=0).reshape(HT, P).T),
        "rb1": np.ascontiguousarray(rb1.reshape(E, LT, P).transpose(0, 2, 1)),
        "rb2": rb2, "rw1": rw1, "rw2": rw2,
        "rb2x": np.ascontiguousarray(
            np.concatenate([rb2, sb2.sum(axis=0)[None]], axis=0)),
    }
    return prep


def _aux_from_stats(mask_sum, sp_sum):
    f_i = mask_sum * (E / (TOP_K * S)) / B
    p_i = sp_sum / T_FULL
    return np.float32(BALANCE_ALPHA * float((f_i * p_i).sum()))


def _perm_h(w):
    """[H, N] -> partition-major [P, HC, N] (contiguous per partition)."""
    return np.ascontiguousarray(w.reshape(-1, P, w.shape[-1]).transpose(1, 0, 2))


def _kernel_ep2(prep):
    zero_bias = (not prep["eb"].any()) and (not prep["rb2x"].any())
    nc1 = _get("s1", _build_stage1, zero_bias)
    flatT_perm = _perm_h(np.ascontiguousarray(prep["flat"].T))  # [P, HC, TF]
    ct_p = _perm_h(prep["ct"])
    s1_p = _perm_h(prep["s1"])
    s2_p = _perm_h(prep["s2"])
    in1 = []
    for c in range(N_CORES):
        xt_c = flatT_perm[:, :, c * T:(c + 1) * T]
        m = {"xta": np.ascontiguousarray(xt_c[:, 0:3]),
             "xtb": np.ascontiguousarray(xt_c[:, 3:6]),
             "xtc": np.ascontiguousarray(xt_c[:, 6:8]),
             "ct": ct_p, "s1": s1_p, "s2": s2_p, "sb1": prep["sb1"]}
        if not zero_bias:
            m["eb"] = prep["eb"]
            m["rb2"] = prep["rb2x"]
        in1.append(m)
    res1 = run_bass_kernel_spmd(nc1, in1, core_ids=list(range(N_CORES)))

    mask_sum = np.zeros(E, dtype=np.float64)
    sp_sum = np.zeros(E, dtype=np.float64)
    gT_full = np.empty((E, T_FULL), dtype=np.float32)
    out = np.empty((T_FULL, H), dtype=np.float32)
    for c in range(N_CORES):
        r = res1.results[c]
        out[c * T:(c + 1) * T] = r["yt"].T
        gT_full[:, c * T:(c + 1) * T] = r["gt_out"]
        mask_sum += r["stats"][:, 0]
        sp_sum += r["stats"][:, 1]
    aux = _aux_from_stats(mask_sum, sp_sum)

    # token dispatch on the device-computed top-k gates; rank experts by
    # load so the low-count slot can use a smaller capacity
    idx = [np.nonzero(gT_full[e] > 0)[0] for e in range(E)]
    order = sorted(range(E), key=lambda e: -len(idx[e]))
    # core c: slot 0 <- rank c (heavy), slot 1 <- rank 15-c (light)
    assign = [(order[c], order[E - 1 - c]) for c in range(N_CORES)]
    rup = lambda n: max(512, -(-n // 256) * 256)
    C1 = rup(max(len(idx[e]) for e, _ in assign))
    C2 = rup(max(len(idx[e]) for _, e in assign))

    w1_perm = np.ascontiguousarray(
        prep["rw1"].reshape(E, HC, P, L).transpose(0, 2, 1, 3))
    w2_perm = np.ascontiguousarray(
        prep["rw2"].reshape(E, LT, P, H).transpose(0, 2, 1, 3))

    nc2 = _get("s2", _build_stage2, C1, C2)
    caps = (C1, C2)
    in2 = []
    for c in range(N_CORES):
        m = {}
        for k in range(EPC):
            e = assign[c][k]
            n = len(idx[e])
            xe = np.zeros((P, HC, caps[k]), dtype=np.float32)
            gevals = np.zeros((1, caps[k]), dtype=np.float32)
            xe[:, :, :n] = flatT_perm[:, :, idx[e]]
            gevals[0, :n] = gT_full[e, idx[e]]
            m[f"xe{k}"] = xe
            m[f"ge{k}"] = gevals
        es = list(assign[c])
        m["w1p"] = np.ascontiguousarray(w1_perm[es])
        m["w2p"] = np.ascontiguousarray(w2_perm[es])
        m["rb1p"] = np.ascontiguousarray(prep["rb1"][es])
        in2.append(m)
    res2 = run_bass_kernel_spmd(nc2, in2, core_ids=list(range(N_CORES)))

    for c in range(N_CORES):
        for k in range(EPC):
            e = assign[c][k]
            n = len(idx[e])
            out[idx[e]] += res2.results[c][f"ye{k}"][:, :n].T
    return out.reshape(B, S, H), aux


def _kernel_dense(prep):
    nc = _get("dense", _build_dense)
    in_maps = []
    for c in range(N_CORES):
        xt_c = np.ascontiguousarray(prep["flat"][c * T:(c + 1) * T].T)
        in_maps.append({
            "xt": xt_c, "ct": prep["ct"], "eb": prep["eb"],
            "w1": prep["rw1"], "w2": prep["rw2"], "s1": prep["s1"],
            "s2": prep["s2"], "rb1": prep["rb1"], "rb2": prep["rb2"],
            "sb1": prep["sb1"], "sb2": prep["sb2"],
        })
    res = run_bass_kernel_spmd(nc, in_maps, core_ids=list(range(N_CORES)))

    out = np.empty((T_FULL, H), dtype=np.float32)
    mask_sum = np.zeros(E, dtype=np.float64)
    sp_sum = np.zeros(E, dtype=np.float64)
    for c in range(N_CORES):
        out[c * T:(c + 1) * T] = res.results[c]["yt"].T
        st = res.results[c]["stats"]
        mask_sum += st[:, 0]
        sp_sum += st[:, 1]
    return out.reshape(B, S, H), _aux_from_stats(mask_sum, sp_sum)


def _build_dense():
    """Single-launch fallback: every core runs all experts on its shard."""
    nc = bacc.Bacc("TRN2", target_bir_lowering=False, debug=False,
                   num_devices=N_CORES)

    xt = nc.dram_tensor("xt", [H, T], F32, kind="ExternalInput").ap()
    ct = nc.dram_tensor("ct", [H, E], F32, kind="ExternalInput").ap()
    eb = nc.dram_tensor("eb", [E, 1], F32, kind="ExternalInput").ap()
    w1 = nc.dram_tensor("w1", [E, H, L], F32, kind="ExternalInput").ap()
    w2 = nc.dram_tensor("w2", [E, L, H], F32, kind="ExternalInput").ap()
    s1 = nc.dram_tensor("s1", [H, L], F32, kind="ExternalInput").ap()
    s2 = nc.dram_tensor("s2", [L, H], F32, kind="ExternalInput").ap()
    rb1 = nc.dram_tensor("rb1", [E, P, LT], F32, kind="ExternalInput").ap()
    rb2 = nc.dram_tensor("rb2", [E, H], F32, kind="ExternalInput").ap()
    sb1 = nc.dram_tensor("sb1", [P, LT], F32, kind="ExternalInput").ap()
    sb2 = nc.dram_tensor("sb2", [P, HT], F32, kind="ExternalInput").ap()
    yt = nc.dram_tensor("yt", [H, T], F32, kind="ExternalOutput").ap()
    stats = nc.dram_tensor("stats", [E, 2], F32, kind="ExternalOutput").ap()

    with tile.TileContext(nc) as tc, ExitStack() as ctx:
        const = ctx.enter_context(tc.tile_pool(name="const", bufs=1))
        xpool = ctx.enter_context(tc.tile_pool(name="xpool", bufs=1))
        work = ctx.enter_context(tc.tile_pool(name="work", bufs=2))
        small = ctx.enter_context(tc.tile_pool(name="small", bufs=4))
        ph_pool = ctx.enter_context(tc.tile_pool(name="ph", bufs=2, space="PSUM"))
        po_pool = ctx.enter_context(tc.tile_pool(name="po", bufs=2, space="PSUM"))
        pm_pool = ctx.enter_context(tc.tile_pool(name="pm", bufs=2, space="PSUM"))
        ps_pool = ctx.enter_context(tc.tile_pool(name="ps", bufs=1, space="PSUM"))

        identity = const.tile([P, P], F32)
        make_identity(nc, identity)
        ones_col = const.tile([P, 1], F32)
        nc.any.memset(ones_col[:], 1.0)
        ones_row = const.tile([1, P], F32)
        nc.any.memset(ones_row[:], 1.0)

        xt_sb = xpool.tile([P, HC, T], F32, tag="x")
        nc.sync.dma_start(xt_sb[:], xt.rearrange("(c p) t -> p c t", p=P))
        ct_sb = const.tile([P, HC, E], F32)
        nc.sync.dma_start(ct_sb[:], ct.rearrange("(c p) e -> p c e", p=P))
        eb_sb = const.tile([E, 1], F32)
        nc.sync.dma_start(eb_sb[:], eb[:])
        rb2_sb = const.tile([E, H], F32)
        nc.sync.dma_start(rb2_sb[:], rb2[:])
        rb1_sb = const.tile([P, E, LT], F32)
        nc.sync.dma_start(rb1_sb[:], rb1.rearrange("e p l -> p e l"))
        sb1_sb = const.tile([P, LT], F32)
        nc.sync.dma_start(sb1_sb[:], sb1[:])
        sb2_sb = const.tile([P, HT], F32)
        nc.sync.dma_start(sb2_sb[:], sb2[:])

        acc = const.tile([P, HT, T], F32)
        gT = const.tile([E, T], F32)

        _router_and_gates(nc, tc, ctx, (small, pm_pool, ps_pool), xt_sb,
                          ct_sb, eb_sb, ones_col, identity, gT, stats)

        for ht in range(HT):
            hsl = slice(ht * P, (ht + 1) * P)
            for half in range(NH):
                csl = slice(half * 512, (half + 1) * 512)
                pc = po_pool.tile([P, 512], F32, tag="po")
                nc.tensor.matmul(pc[:], rb2_sb[:, hsl], gT[:, csl],
                                 start=True, stop=True)
                nc.vector.tensor_add(acc[:, ht, csl], xt_sb[:, ht, csl], pc[:])
                nc.vector.tensor_scalar_add(acc[:, ht, csl], acc[:, ht, csl],
                                            sb2_sb[:, ht:ht + 1])

        xt_r = xpool.tile([P, HC, T], F32R, tag="x")
        nc.gpsimd.dma_start(xt_r[:], xt.rearrange("(c p) t -> p c t", p=P))

        for e in range(E + 1):
            shared = (e == E)
            w1_sb = work.tile([P, HC, L], F32R, tag="w1")
            w2_sb = work.tile([P, LT, H], F32R, tag="w2")
            if shared:
                nc.gpsimd.dma_start(w1_sb[:], s1.rearrange("(c p) l -> p c l", p=P))
                nc.gpsimd.dma_start(w2_sb[:], s2.rearrange("(c p) h -> p c h", p=P))
            else:
                nc.gpsimd.dma_start(w1_sb[:], w1[e].rearrange("(c p) l -> p c l", p=P))
                nc.gpsimd.dma_start(w2_sb[:], w2[e].rearrange("(c p) h -> p c h", p=P))

            if not shared:
                gb_sb = work.tile([P, T], F32, tag="gb")
                g_row = work.tile([1, T], F32, tag="g_row")
                nc.sync.dma_start(g_row[:], gT[e:e + 1, :])
                for half in range(NH):
                    csl = slice(half * 512, (half + 1) * 512)
                    pgb = pm_pool.tile([P, 512], F32, tag="pt")
                    nc.tensor.matmul(pgb[:], ones_row[:], g_row[0:1, csl],
                                     start=True, stop=True)
                    nc.vector.tensor_copy(gb_sb[:, csl], pgb[:])

            z_sb = work.tile([P, LT, T], F32R, tag="z")
            for lt in range(LT):
                lsl = slice(lt * P, (lt + 1) * P)
                for half in range(NH):
                    csl = slice(half * 512, (half + 1) * 512)
                    pht = ph_pool.tile([P, 512], F32, tag="ph")
                    for hc in range(HC):
                        nc.tensor.matmul(pht[:], w1_sb[:, hc, lsl],
                                         xt_r[:, hc, csl],
                                         start=(hc == 0), stop=(hc == HC - 1))
                    bias_ap = (sb1_sb if shared else rb1_sb[:, e, :])[:, lt:lt + 1]
                    nc.scalar.activation(z_sb[:, lt, csl], pht[:], AF.Silu,
                                         bias=bias_ap)
                    if not shared:
                        nc.vector.tensor_mul(z_sb[:, lt, csl], z_sb[:, lt, csl],
                                             gb_sb[:, csl])

            for ht in range(HT):
                hsl = slice(ht * P, (ht + 1) * P)
                for half in range(NH):
                    csl = slice(half * 512, (half + 1) * 512)
                    pot = po_pool.tile([P, 512], F32, tag="po")
                    for lc in range(LT):
                        nc.tensor.matmul(pot[:], w2_sb[:, lc, hsl],
                                         z_sb[:, lc, csl],
                                         start=(lc == 0), stop=(lc == LT - 1))
                    nc.vector.tensor_add(acc[:, ht, csl], acc[:, ht, csl], pot[:])

        for ht in range(HT):
            nc.sync.dma_start(yt[ht * P:(ht + 1) * P, :], acc[:, ht, :])

    nc.compile()
    return nc


def kernel(hidden_states, expert_centroids, expert_biases, sw1, sb1, sw2, sb2,
           rw1, rb1, rw2, rb2, **_ignored):
    args = [np.asarray(a, dtype=np.float32) for a in
            (hidden_states, expert_centroids, expert_biases, sw1, sb1, sw2,
             sb2, rw1, rb1, rw2, rb2)]
    prep = _prep_host(args)
    strategy = os.environ.get("MOE_STRATEGY", "ep2")
    if strategy == "dense":
        return _kernel_dense(prep)
    return _kernel_ep2(prep)
